# revision 52
# baseline (speedup 1.0000x reference)
"""ConvSE3 (SE(3)-equivariant graph conv) Trainium2 kernel, 8-core SPMD.

Strategy:
  - Shard (b, n): 4 batches x 256 nodes = 1024 rows -> 8 cores x 128 nodes.
    Each core: 128 nodes x 32 neighbors = 4096 edges (32 chunks of 128).
  - Host: gathers neighbor features, folds LayerNorm-1 analytically into a
    K=3 matmul (input is a per-edge scalar), centers w2/b2 so LN2's mean
    subtraction is free, computes masked-mean weights, packs per-core arrays.
  - Device per pair (di,do) of degree pairs:
      mm1 (K=3) -> relu -> mm2 (K=128) -> LN2 var via PE ones-matmuls ->
      rsqrt (ACT) -> relu(v2c*g2) (ACT, r2>0 folds past relu) -> mm3 -> R in
      PSUM -> per-edge einsum on DVE (edge-major, broadcast APs + segmented
      tensor_reduce) -> masked k-mean as PE matmul with block-diagonal
      per-edge weights (r2 and basis00 folded into those weights).
  - Host: adds self-interaction (tiny einsum) and reassembles full outputs.

Assumes be2 == 0, b3 == 0 and b2 - mean(b2) == 0 on the fast path (true for
this module's init); otherwise falls back to an exact host computation.
"""

import sys
import numpy as np

sys.path.insert(0, "/opt/trn_rl_repo")

B, N, K, M = 4, 256, 32, 16
MID = 128
EPS = 1e-5
NCORES = 8
NODES_PER_CORE = N // 2          # 128
EDGES_PER_CORE = NODES_PER_CORE * K   # 4096
NCHUNK = EDGES_PER_CORE // 128   # 32
NTILE = EDGES_PER_CORE // 512    # 8

# pair order: (di, do) with f = min(2di+1, 2do+1), p = 2do+1, q = 2di+1
PAIRS = [("0", "0"), ("0", "1"), ("1", "0"), ("1", "1")]
PAIR_KEYS = ["0,0", "0,1", "1,0", "1,1"]
PF = [1, 1, 1, 3]                 # f per pair
PP = [1, 3, 1, 3]                 # p per pair
PQ = [1, 1, 3, 3]                 # q per pair
W3_OFF = [0, 256, 512, 768]       # offsets in concatenated w3 [128, 1536]
W3_N = [256, 256, 256, 768]

_CACHE = {}
LINEARIZE = False


def _host_forward(x0, x1, rel_dist, basis, params, neighbor_indices, neighbor_masks):
    """Exact numpy mirror of the jax reference (fallback path)."""
    def ln(x, g, b):
        mu = x.mean(-1, keepdims=True)
        var = ((x - mu) ** 2).mean(-1, keepdims=True)
        return (x - mu) / np.sqrt(var + EPS) * g + b

    def radial(feat, p, o, i, f):
        h = np.maximum(ln(feat @ p["w1"] + p["b1"], p["g1"], p["be1"]), 0)
        h = np.maximum(ln(h @ p["w2"] + p["b2"], p["g2"], p["be2"]), 0)
        y = h @ p["w3"] + p["b3"]
        return y.reshape(y.shape[:-1] + (o, i, f))

    ORD = {"0": 1, "1": 3}
    b, n, k = neighbor_indices.shape
    m = x0.shape[2]
    feat = rel_dist[..., None]
    bidx = np.arange(b)[:, None, None]
    xg = {"0": x0[bidx, neighbor_indices], "1": x1[bidx, neighbor_indices]}
    outs = {}
    for do in ("0", "1"):
        acc = 0.0
        for di in ("0", "1"):
            pk = di + "," + do
            f = min(ORD[di], ORD[do])
            R = radial(feat, params["rp"][pk], m, m, f)
            acc = acc + np.einsum("bnkoif,bnkpqf,bnkiq->bnkop", R, basis[pk], xg[di])
        w = neighbor_masks[..., None, None].astype(x0.dtype)
        outs[do] = (acc * w).sum(2) / np.clip(w.sum(2), 1e-5, None)
    out0 = outs["0"] + np.einsum("bndm,de->bnem", x0, params["w_si0"])
    out1 = outs["1"] + np.einsum("bndm,de->bnem", x1, params["w_si1"])
    return out0.astype(np.float32), out1.astype(np.float32)


def _build_bass():
    """Build the 8-core SPMD Bass program (same program, per-core inputs)."""
    import concourse.bass as bass
    import concourse.mybir as mybir
    from concourse import bacc, tile

    dt = mybir.dt.float32
    AF = mybir.ActivationFunctionType
    ALU = mybir.AluOpType
    AX = mybir.AxisListType

    # Bacc (not raw Bass): its compile() runs move_matmul_waits_to_ldweights
    # and generate_event_semaphores, which legalize multi-wait instructions
    # for the TRN2 sync-struct limits.
    nc = bacc.Bacc(None, target_bir_lowering=False, debug=False)

    # ---- DRAM I/O (per-core shapes) ----
    # big3 = u [3, 4*4096] ++ w1eff [3, 512]  (one DMA -> one queue sem)
    big3_d = nc.dram_tensor("big3", [3, 4 * EDGES_PER_CORE + 512], dt,
                            kind="ExternalInput")
    # bigw = w2p [128,512] ++ w3cat(g2-folded) [128,1536] ++ ones/128 [128,1]
    bigw_d = nc.dram_tensor("bigw", [128, 2049], dt, kind="ExternalInput")
    # edata: per-edge chunk-major data, 102 f32/edge:
    #   xg0 0:16 | xg1 16:64 | b00 64:65 | b01 65:68 | b10 68:71 |
    #   b11 71:98 | wblk 98:102
    edata_d = nc.dram_tensor("edata", [128, NCHUNK * 102], dt,
                             kind="ExternalInput")
    out_d = nc.dram_tensor("outd", [4, NCHUNK * 64], dt, kind="ExternalOutput")

    # ---- preamble (raw bass, before Tile): load all constants, then an
    # all-engine barrier. Tile then sees no DMA producers, so compute
    # instructions never need DMA-queue waits (PE matmul allows only one
    # wait; multi-wait instructions fail walrus codegen).
    big3_t = nc.alloc_sbuf_tensor("big3_sb", [3, 4 * EDGES_PER_CORE + 512], dt)
    bigw_t = nc.alloc_sbuf_tensor("bigw_sb", [128, 2049], dt)
    edata_t = nc.alloc_sbuf_tensor("edata_sb", [128, NCHUNK * 102], dt)
    eps_t = nc.alloc_sbuf_tensor("eps_sb", [128, 1], dt)
    from contextlib import ExitStack
    es = ExitStack()
    ldsem = es.enter_context(nc.semaphore())
    nc.sync.dma_start(big3_t.ap(), big3_d[:]).then_inc(ldsem, 16)
    nc.sync.dma_start(bigw_t.ap(), bigw_d[:]).then_inc(ldsem, 16)
    nc.sync.dma_start(edata_t.ap(), edata_d[:]).then_inc(ldsem, 16)
    nc.gpsimd.memset(eps_t.ap(), EPS)
    for eng in (nc.tensor, nc.vector, nc.scalar, nc.gpsimd, nc.sync):
        eng.wait_ge(ldsem, 48)
    nc.all_engine_barrier()

    with tile.TileContext(nc, linearize=LINEARIZE) as tc:
        with (
            tc.tile_pool(name="const", bufs=1) as cpool,
            tc.tile_pool(name="work", bufs=3) as wpool,
            tc.tile_pool(name="small", bufs=4) as spool,
            tc.tile_pool(name="mlp", bufs=2, space=bass.MemorySpace.PSUM) as mlppool,
            tc.tile_pool(name="rps", bufs=1, space=bass.MemorySpace.PSUM) as rpool,
            tc.tile_pool(name="stat", bufs=1, space=bass.MemorySpace.PSUM) as statpool,
            tc.tile_pool(name="nout", bufs=2, space=bass.MemorySpace.PSUM) as npool,
        ):
            # ---- views onto the preamble-loaded constants ----
            outbuf = cpool.tile([4, NCHUNK * 64], dt)
            big3_sb = big3_t.ap()
            bigw_sb = bigw_t.ap()
            edata_sb = edata_t.ap().rearrange("p (a b) -> p a b", b=102)
            eps_sb = eps_t.ap()

            UOFF = 4 * EDGES_PER_CORE       # w1eff offset inside big3
            w3_sb = bigw_sb[:, 512:2048]
            ones_sb = bigw_sb[:, 2048:2049]   # holds 1/128
            xg0_sb = edata_sb[:, :, 0:16]
            xg1_sb = edata_sb[:, :, 16:64]
            b00_sb = edata_sb[:, :, 64:65]
            b01_sb = edata_sb[:, :, 65:68]
            b10_sb = edata_sb[:, :, 68:71]
            b11_sb = edata_sb[:, :, 71:98]
            wblk_sb = edata_sb[:, :, 98:102]



            for t in range(NTILE):
                e0 = t * 512
                # per-pair SBUF intermediates for this tile
                h2s = []
                r_all = spool.tile([128, 16], dt, tag="r_all")  # [P*4 + jj]
                for P in range(4):
                    psA = mlppool.tile([128, 512], dt, tag="ps")
                    psV = mlppool.tile([128, 512], dt, tag="ps")
                    h1 = wpool.tile([128, 512], dt, tag="h1")
                    sq = wpool.tile([128, 512], dt, tag="sq")
                    h2 = wpool.tile([128, 512], dt, tag=f"h2_{P}")
                    psVar = statpool.tile([128, 4], dt, tag="psVar")

                    # mm1: [3,128].T @ [3,512] -> [128,512]
                    nc.tensor.matmul(
                        psA[:],
                        big3_sb[:, UOFF + P * 128: UOFF + (P + 1) * 128],
                        big3_sb[:, P * EDGES_PER_CORE + e0:
                                P * EDGES_PER_CORE + e0 + 512],
                        start=True, stop=True,
                    )
                    nc.scalar.activation(h1[:], psA[:], AF.Relu)
                    # mm2 (w2 pre-centered): [128,128].T @ [128,512]
                    nc.tensor.matmul(
                        psV[:], bigw_sb[:, P * 128:(P + 1) * 128], h1[:],
                        start=True, stop=True,
                    )
                    # LN2 variance: square then per-chunk ones-matmul column sums
                    nc.scalar.activation(sq[:], psV[:], AF.Square)
                    for jj in range(4):
                        nc.tensor.matmul(
                            psVar[:, jj:jj + 1],
                            sq[:, jj * 128:(jj + 1) * 128],
                            ones_sb[:],
                            start=True, stop=True,
                        )
                    # r2 = rsqrt(var/128 + eps)  (edge-major [128,4])
                    sd = spool.tile([128, 4], dt, tag="sd")
                    nc.scalar.activation(sd[:], psVar[:], AF.Sqrt, bias=eps_sb[:])
                    if P == 0:
                        # fold basis00 per-edge scalar into pair00's r
                        rb = spool.tile([128, 4], dt, tag="rb")
                        nc.vector.reciprocal(rb[:], sd[:])
                        nc.vector.tensor_tensor(
                            r_all[:, 0:4], rb[:], b00_sb[:, t * 4:t * 4 + 4, 0],
                            op=ALU.mult,
                        )
                    else:
                        nc.vector.reciprocal(r_all[:, P * 4:(P + 1) * 4], sd[:])
                    # h2' = relu(v2c); g2 (>0) and r2 (>0) fold into w3 / k-mean
                    nc.scalar.activation(h2[:], psV[:], AF.Relu)
                    h2s.append(h2)

                # ---- per chunk: mm3 + einsum + masked k-mean ----
                for jj in range(4):
                    j = t * 4 + jj
                    psRa = rpool.tile([128, 512], dt, tag="psRa")   # R00|R01
                    psRb = rpool.tile([128, 1024], dt, tag="psRb")  # R11|R10
                    for P in range(2):
                        nc.tensor.matmul(
                            psRa[:, P * 256:(P + 1) * 256],
                            h2s[P][:, jj * 128:(jj + 1) * 128],
                            w3_sb[:, W3_OFF[P]:W3_OFF[P] + 256],
                            start=True, stop=True,
                        )
                    nc.tensor.matmul(
                        psRb[:, 0:512],
                        h2s[3][:, jj * 128:(jj + 1) * 128],
                        w3_sb[:, 768:1280],
                        start=True, stop=True,
                    )
                    nc.tensor.matmul(
                        psRb[:, 512:768],
                        h2s[3][:, jj * 128:(jj + 1) * 128],
                        w3_sb[:, 1280:1536],
                        start=True, stop=True,
                    )
                    nc.tensor.matmul(
                        psRb[:, 768:1024],
                        h2s[2][:, jj * 128:(jj + 1) * 128],
                        w3_sb[:, W3_OFF[2]:W3_OFF[2] + 256],
                        start=True, stop=True,
                    )

                    psN = npool.tile([4, 64], dt, tag="psN")

                    # --- pairs 00 & 01 share: s[e, P, o] = sum_i R[e,(P,o,i)]*xg0[e,i]
                    pbuf = wpool.tile([128, 512], dt, tag="pbuf")
                    s01 = spool.tile([128, 32], dt, tag="s01")
                    xg0j = xg0_sb[:, j, :]  # [128, 16]
                    nc.vector.tensor_tensor(
                        pbuf[:].rearrange("e (a i) -> e a i", i=16),
                        psRa[:, 0:512].rearrange("e (a i) -> e a i", i=16),
                        xg0j.unsqueeze(1).broadcast_to([128, 32, 16]),
                        op=ALU.mult,
                    )
                    nc.vector.tensor_reduce(
                        s01[:],
                        pbuf[:].rearrange("e (a i) -> e a i", i=16),
                        axis=AX.X, op=ALU.add,
                    )
                    # pair01 output: outP01[e, o, p] = s01[e, 16+o] * b01[e, p]
                    o01 = spool.tile([128, 48], dt, tag="o01")
                    nc.vector.tensor_tensor(
                        o01[:].rearrange("e (o p) -> e o p", p=3),
                        s01[:, 16:32].unsqueeze(2).broadcast_to([128, 16, 3]),
                        b01_sb[:, j, :].unsqueeze(1).broadcast_to([128, 16, 3]),
                        op=ALU.mult,
                    )

                    # --- pair10: t10[e,i] = sum_q xg1[e,i,q]*b10[e,q]
                    tb = spool.tile([128, 48], dt, tag="tb")
                    t10 = spool.tile([128, 16], dt, tag="t10")
                    o10 = spool.tile([128, 16], dt, tag="o10")
                    nc.vector.tensor_tensor(
                        tb[:].rearrange("e (i q) -> e i q", q=3),
                        xg1_sb[:, j, :].rearrange("e (i q) -> e i q", q=3),
                        b10_sb[:, j, :].unsqueeze(1).broadcast_to([128, 16, 3]),
                        op=ALU.mult,
                    )
                    nc.vector.tensor_reduce(
                        t10[:], tb[:].rearrange("e (i q) -> e i q", q=3),
                        axis=AX.X, op=ALU.add,
                    )
                    nc.vector.tensor_tensor(
                        pbuf[:, 0:256].rearrange("e (o i) -> e o i", i=16),
                        psRb[:, 768:1024].rearrange("e (o i) -> e o i", i=16),
                        t10[:].unsqueeze(1).broadcast_to([128, 16, 16]),
                        op=ALU.mult,
                    )
                    nc.vector.tensor_reduce(
                        o10[:], pbuf[:, 0:256].rearrange("e (o i) -> e o i", i=16),
                        axis=AX.X, op=ALU.add,
                    )

                    # --- pair11 ---
                    tbuf = wpool.tile([128, 432], dt, tag="tbuf")
                    t11 = spool.tile([128, 144], dt, tag="t11")
                    p11 = wpool.tile([128, 2304], dt, tag="p11")
                    o11 = spool.tile([128, 48], dt, tag="o11")
                    # T[e,(i,f,p)] = sum_q xg1[e,i,q] * b11[e,(p,q,f)]
                    # (split by p: HW APs allow at most 3 free dims)
                    tb5 = tbuf[:].rearrange("e (i f p q) -> e i f p q",
                                            f=3, p=3, q=3)
                    xg15 = (xg1_sb[:, j, :]
                            .rearrange("e (i q) -> e i q", q=3)
                            .unsqueeze(2)
                            .broadcast_to([128, 16, 3, 3]))
                    b115 = (b11_sb[:, j, :]
                            .rearrange("e (p q f) -> e p q f", q=3, f=3)
                            .transpose([0, 1, 3, 2]))  # -> [e, p, f, q]
                    for pp in range(3):
                        nc.vector.tensor_tensor(
                            tb5[:, :, :, pp, :],
                            xg15,
                            b115[:, pp, :, :].unsqueeze(1)
                            .broadcast_to([128, 16, 3, 3]),
                            op=ALU.mult,
                        )
                    nc.vector.tensor_reduce(
                        t11[:],
                        tbuf[:].rearrange("e (ifp q) -> e ifp q", q=3),
                        axis=AX.X, op=ALU.add,
                    )
                    # prod5[e, o,p,i,f] = R11[e,(o,i,f)] * T[e,(i,f,p)]
                    # (split by p: HW APs allow at most 3 free dims)
                    p115 = p11[:].rearrange("e (o p i f) -> e o p i f",
                                            o=16, p=3, i=16)
                    r115 = psRb[:, 0:768].rearrange("e (o i f) -> e o i f",
                                                    o=16, i=16)
                    t115 = t11[:].rearrange("e (i f p) -> e i f p", i=16, f=3)
                    for pp in range(3):
                        nc.vector.tensor_tensor(
                            p115[:, :, pp, :, :],
                            r115,
                            t115[:, :, :, pp].unsqueeze(1)
                            .broadcast_to([128, 16, 16, 3]),
                            op=ALU.mult,
                        )
                    nc.vector.tensor_reduce(
                        o11[:],
                        p11[:].rearrange("e (op i_f) -> e op i_f", i_f=48),
                        axis=AX.X, op=ALU.add,
                    )

                    # --- masked k-mean via block-diag PE matmuls ---
                    # wr_all[e, P, nn] = wblk[e, nn] * r_all[e, P*4+jj]
                    wr = spool.tile([128, 4, 4], dt, tag="wr")
                    nc.vector.tensor_tensor(
                        wr[:],
                        wblk_sb[:, j, :].unsqueeze(1).broadcast_to([128, 4, 4]),
                        r_all[:].rearrange("e (P c) -> e P c", c=4)[:, :, jj:jj + 1]
                        .broadcast_to([128, 4, 4]),
                        op=ALU.mult,
                    )
                    # do=0: pairs 00 (s01[:, :16]) and 10
                    nc.tensor.matmul(psN[:, 0:16], wr[:, 0, :], s01[:, 0:16],
                                     start=True, stop=False)
                    nc.tensor.matmul(psN[:, 0:16], wr[:, 2, :], o10[:],
                                     start=False, stop=True)
                    # do=1: pairs 01 and 11
                    nc.tensor.matmul(psN[:, 16:64], wr[:, 1, :], o01[:],
                                     start=True, stop=False)
                    nc.tensor.matmul(psN[:, 16:64], wr[:, 3, :], o11[:],
                                     start=False, stop=True)
                    # DVE copy (not ACT) keeps k-mean matmuls at one wait each
                    nc.vector.tensor_copy(outbuf[:, j * 64:(j + 1) * 64], psN[:])

            nc.sync.dma_start(out_d[:], outbuf[:])

    es.close()
    nc.compile()
    return nc


def _prep_core_inputs(x0, x1, rel_dist, basis, params, nidx, nmask):
    """Host-side folding + packing. Returns list of 8 in_maps."""
    f32 = np.float32
    # --- per-pair folded weights (shared across cores) ---
    w1eff = np.zeros((3, 4 * 128), f32)
    w2p = np.zeros((128, 4 * 128), f32)
    g2p = np.zeros((128, 4), f32)
    w3cat = np.zeros((128, 1536), f32)
    ones_col = np.full((128, 1), 1.0 / 128.0, f32)
    lnco = []  # (m20, m11, m02, mw, mb) per pair for u computation
    for P, pk in enumerate(PAIR_KEYS):
        rp = {k: np.asarray(v, f32) for k, v in params["rp"][pk].items()}
        w1 = rp["w1"][0]          # [128]
        b1 = rp["b1"]
        mw, mb = w1.mean(), b1.mean()
        wc, bc = w1 - mw, b1 - mb
        m20 = (wc * wc).mean()
        m11 = (wc * bc).mean()
        m02 = (bc * bc).mean()
        lnco.append((m20, m11, m02))
        w1eff[0, P * 128:(P + 1) * 128] = wc * rp["g1"]
        w1eff[1, P * 128:(P + 1) * 128] = bc * rp["g1"]
        w1eff[2, P * 128:(P + 1) * 128] = rp["be1"]
        w2p[:, P * 128:(P + 1) * 128] = rp["w2"] - rp["w2"].mean(1, keepdims=True)
        g2p[:, P] = rp["g2"]
        # w3 natural layout [128, o*i*f] == [c, (o,i,f)]; g2 (>0) folded in rows
        w3cat[:, W3_OFF[P]:W3_OFF[P] + W3_N[P]] = rp["g2"][:, None] * rp["w3"]
    # fast-path validity
    fast = True
    for pk in PAIR_KEYS:
        rp = params["rp"][pk]
        b2c = np.asarray(rp["b2"], f32)
        b2c = b2c - b2c.mean()
        if (np.abs(b2c).max() > 0 or np.abs(np.asarray(rp["be2"])).max() > 0
                or np.abs(np.asarray(rp["b3"])).max() > 0
                or np.asarray(rp["g2"]).min() <= 0):
            fast = False
    if not fast:
        return None

    nidx = np.asarray(nidx)
    nmask_f = np.asarray(nmask).astype(f32)
    denom = np.clip(nmask_f.sum(2), 1e-5, None)      # [B, N]
    wtil = nmask_f / denom[:, :, None]               # [B, N, K]

    in_maps = []
    for c in range(NCORES):
        b = c // 2
        n0 = (c % 2) * NODES_PER_CORE
        ns = slice(n0, n0 + NODES_PER_CORE)
        rd = np.asarray(rel_dist[b, ns], f32)        # [128, 32]
        idx = nidx[b, ns]                            # [128, 32]
        # edge order: e = n_local*32 + k ; chunk j = e//128 ; lane = e%128
        x = rd.reshape(-1)                           # [4096]
        u = np.zeros((3, 4 * EDGES_PER_CORE), f32)
        for P in range(4):
            m20, m11, m02 = lnco[P]
            r1 = 1.0 / np.sqrt(x * x * m20 + 2 * x * m11 + m02 + EPS)
            u[0, P * EDGES_PER_CORE:(P + 1) * EDGES_PER_CORE] = x * r1
            u[1, P * EDGES_PER_CORE:(P + 1) * EDGES_PER_CORE] = r1
            u[2, P * EDGES_PER_CORE:(P + 1) * EDGES_PER_CORE] = 1.0

        flat_idx = idx.reshape(-1)                   # [4096] node ids
        xg0 = np.asarray(x0[b], f32)[flat_idx, :, 0]          # [4096, 16]
        xg1 = np.asarray(x1[b], f32)[flat_idx].reshape(-1, 48)  # [4096, 16*3]
        bas = {pk: np.asarray(basis[pk][b, ns], f32).reshape(EDGES_PER_CORE, -1)
               for pk in PAIR_KEYS}
        wt = wtil[b, ns].reshape(-1)                 # [4096]
        # block-diag k-mean weights: wblk[e, nn] = wt[e] if (e%128)//32==nn
        wblk = np.zeros((EDGES_PER_CORE, 4), f32)
        lane = np.arange(EDGES_PER_CORE) % 128
        wblk[np.arange(EDGES_PER_CORE), lane // 32] = wt

        def pack(a):  # [4096, F] -> [128, NCHUNK*F] with [lane, chunk, F]
            F = a.shape[1]
            return np.ascontiguousarray(
                a.reshape(NCHUNK, 128, F).transpose(1, 0, 2).reshape(128, NCHUNK * F)
            )

        edata = np.concatenate(
            [xg0, xg1, bas["0,0"], bas["0,1"], bas["1,0"], bas["1,1"], wblk],
            axis=1)  # [4096, 102]
        in_maps.append({
            "big3": np.concatenate([u, w1eff], axis=1),
            "bigw": np.concatenate([w2p, w3cat, ones_col], axis=1),
            "edata": pack(edata),
        })
    return in_maps


def kernel(x0, x1, rel_dist, basis00, basis01, basis10, basis11, params,
           neighbor_indices, neighbor_masks):
    x0 = np.asarray(x0, np.float32)
    x1 = np.asarray(x1, np.float32)
    basis = {"0,0": np.asarray(basis00), "0,1": np.asarray(basis01),
             "1,0": np.asarray(basis10), "1,1": np.asarray(basis11)}

    in_maps = _prep_core_inputs(x0, x1, rel_dist, basis, params,
                                neighbor_indices, neighbor_masks)
    if in_maps is None:  # general-params fallback (never taken for this module)
        return _host_forward(x0, x1, np.asarray(rel_dist, np.float32), basis,
                             params, np.asarray(neighbor_indices),
                             np.asarray(neighbor_masks))

    if "nc" not in _CACHE:
        _CACHE["nc"] = _build_bass()
    nc = _CACHE["nc"]

    from concourse import bass_utils
    res = bass_utils.run_bass_kernel_spmd(nc, in_maps, core_ids=list(range(NCORES)))
    outs = res.results

    # reassemble: outd [4, NCHUNK*64] -> per core nodes
    out0 = np.zeros((B, N, 16), np.float32)
    out1 = np.zeros((B, N, 48), np.float32)
    for c in range(NCORES):
        od = outs[c]["outd"].reshape(4, NCHUNK, 64)   # [nn, chunk, 64]
        b = c // 2
        n0 = (c % 2) * NODES_PER_CORE
        # node = n0 + chunk*4 + nn
        o = od.transpose(1, 0, 2)                     # [chunk, nn, 64]
        o = o.reshape(NODES_PER_CORE, 64)
        out0[b, n0:n0 + NODES_PER_CORE] = o[:, 0:16]
        out1[b, n0:n0 + NODES_PER_CORE] = o[:, 16:64]

    out0 = out0.reshape(B, N, 16, 1)
    out1 = out1.reshape(B, N, 16, 3)
    # self-interaction (host, tiny)
    w_si0 = np.asarray(params["w_si0"], np.float32)
    w_si1 = np.asarray(params["w_si1"], np.float32)
    out0 = out0 + np.einsum("bndm,de->bnem", x0, w_si0)
    out1 = out1 + np.einsum("bndm,de->bnem", x1, w_si1)
    return out0.astype(np.float32), out1.astype(np.float32)


# revision 53
# speedup vs baseline: 1.0250x; 1.0250x over previous
"""ConvSE3 (SE(3)-equivariant graph conv) Trainium2 kernel, 8-core SPMD.

Strategy:
  - Shard (b, n): 4 batches x 256 nodes = 1024 rows -> 8 cores x 128 nodes.
    Each core: 128 nodes x 32 neighbors = 4096 edges (32 chunks of 128).
  - Host: gathers neighbor features, folds LayerNorm-1 analytically into a
    K=3 matmul (input is a per-edge scalar), centers w2/b2 so LN2's mean
    subtraction is free, computes masked-mean weights, packs per-core arrays.
  - Device per pair (di,do) of degree pairs:
      mm1 (K=3) -> relu -> mm2 (K=128) -> LN2 var via PE ones-matmuls ->
      rsqrt (ACT) -> relu(v2c*g2) (ACT, r2>0 folds past relu) -> mm3 -> R in
      PSUM -> per-edge einsum on DVE (edge-major, broadcast APs + segmented
      tensor_reduce) -> masked k-mean as PE matmul with block-diagonal
      per-edge weights (r2 and basis00 folded into those weights).
  - Host: adds self-interaction (tiny einsum) and reassembles full outputs.

Assumes be2 == 0, b3 == 0 and b2 - mean(b2) == 0 on the fast path (true for
this module's init); otherwise falls back to an exact host computation.
"""

import sys
import numpy as np

sys.path.insert(0, "/opt/trn_rl_repo")

B, N, K, M = 4, 256, 32, 16
MID = 128
EPS = 1e-5
NCORES = 8
NODES_PER_CORE = N // 2          # 128
EDGES_PER_CORE = NODES_PER_CORE * K   # 4096
NCHUNK = EDGES_PER_CORE // 128   # 32
NTILE = EDGES_PER_CORE // 512    # 8

# pair order: (di, do) with f = min(2di+1, 2do+1), p = 2do+1, q = 2di+1
PAIRS = [("0", "0"), ("0", "1"), ("1", "0"), ("1", "1")]
PAIR_KEYS = ["0,0", "0,1", "1,0", "1,1"]
PF = [1, 1, 1, 3]                 # f per pair
PP = [1, 3, 1, 3]                 # p per pair
PQ = [1, 1, 3, 3]                 # q per pair
W3_OFF = [0, 256, 512, 768]       # offsets in concatenated w3 [128, 1536]
W3_N = [256, 256, 256, 768]

_CACHE = {}
LINEARIZE = False


def _host_forward(x0, x1, rel_dist, basis, params, neighbor_indices, neighbor_masks):
    """Exact numpy mirror of the jax reference (fallback path)."""
    def ln(x, g, b):
        mu = x.mean(-1, keepdims=True)
        var = ((x - mu) ** 2).mean(-1, keepdims=True)
        return (x - mu) / np.sqrt(var + EPS) * g + b

    def radial(feat, p, o, i, f):
        h = np.maximum(ln(feat @ p["w1"] + p["b1"], p["g1"], p["be1"]), 0)
        h = np.maximum(ln(h @ p["w2"] + p["b2"], p["g2"], p["be2"]), 0)
        y = h @ p["w3"] + p["b3"]
        return y.reshape(y.shape[:-1] + (o, i, f))

    ORD = {"0": 1, "1": 3}
    b, n, k = neighbor_indices.shape
    m = x0.shape[2]
    feat = rel_dist[..., None]
    bidx = np.arange(b)[:, None, None]
    xg = {"0": x0[bidx, neighbor_indices], "1": x1[bidx, neighbor_indices]}
    outs = {}
    for do in ("0", "1"):
        acc = 0.0
        for di in ("0", "1"):
            pk = di + "," + do
            f = min(ORD[di], ORD[do])
            R = radial(feat, params["rp"][pk], m, m, f)
            acc = acc + np.einsum("bnkoif,bnkpqf,bnkiq->bnkop", R, basis[pk], xg[di])
        w = neighbor_masks[..., None, None].astype(x0.dtype)
        outs[do] = (acc * w).sum(2) / np.clip(w.sum(2), 1e-5, None)
    out0 = outs["0"] + np.einsum("bndm,de->bnem", x0, params["w_si0"])
    out1 = outs["1"] + np.einsum("bndm,de->bnem", x1, params["w_si1"])
    return out0.astype(np.float32), out1.astype(np.float32)


def _build_bass():
    """Build the 8-core SPMD Bass program (same program, per-core inputs)."""
    import concourse.bass as bass
    import concourse.mybir as mybir
    from concourse import bacc, tile

    dt = mybir.dt.float32
    AF = mybir.ActivationFunctionType
    ALU = mybir.AluOpType
    AX = mybir.AxisListType

    # Bacc (not raw Bass): its compile() runs move_matmul_waits_to_ldweights
    # and generate_event_semaphores, which legalize multi-wait instructions
    # for the TRN2 sync-struct limits.
    nc = bacc.Bacc(None, target_bir_lowering=False, debug=False)

    # ---- DRAM I/O (per-core shapes) ----
    # big3 = u [3, 4*4096] ++ w1eff [3, 512]  (one DMA -> one queue sem)
    big3_d = nc.dram_tensor("big3", [3, 4 * EDGES_PER_CORE + 512], dt,
                            kind="ExternalInput")
    # bigw = w2p [128,512] ++ w3cat(g2-folded) [128,1536] ++ ones/128 [128,1]
    bigw_d = nc.dram_tensor("bigw", [128, 2049], dt, kind="ExternalInput")
    # edata: per-edge chunk-major data, 102 f32/edge:
    #   xg0 0:16 | xg1 16:64 | b00 64:65 | b01 65:68 | b10 68:71 |
    #   b11 71:98 | wblk 98:102
    edata_d = nc.dram_tensor("edata", [128, NCHUNK * 102], dt,
                             kind="ExternalInput")
    out_d = nc.dram_tensor("outd", [4, NCHUNK * 64], dt, kind="ExternalOutput")

    # ---- preamble (raw bass, before Tile): load all constants, then an
    # all-engine barrier. Tile then sees no DMA producers, so compute
    # instructions never need DMA-queue waits (PE matmul allows only one
    # wait; multi-wait instructions fail walrus codegen).
    big3_t = nc.alloc_sbuf_tensor("big3_sb", [3, 4 * EDGES_PER_CORE + 512], dt)
    bigw_t = nc.alloc_sbuf_tensor("bigw_sb", [128, 2049], dt)
    edata_t = nc.alloc_sbuf_tensor("edata_sb", [128, NCHUNK * 102], dt)
    eps_t = nc.alloc_sbuf_tensor("eps_sb", [128, 1], dt)
    from contextlib import ExitStack
    es = ExitStack()
    ldsem = es.enter_context(nc.semaphore())
    nc.sync.dma_start(big3_t.ap(), big3_d[:]).then_inc(ldsem, 16)
    nc.sync.dma_start(bigw_t.ap(), bigw_d[:]).then_inc(ldsem, 16)
    nc.sync.dma_start(edata_t.ap(), edata_d[:]).then_inc(ldsem, 16)
    nc.gpsimd.memset(eps_t.ap(), EPS)
    for eng in (nc.tensor, nc.vector, nc.scalar, nc.gpsimd, nc.sync):
        eng.wait_ge(ldsem, 48)
    nc.all_engine_barrier()

    with tile.TileContext(nc, linearize=LINEARIZE) as tc:
        with (
            tc.tile_pool(name="const", bufs=1) as cpool,
            tc.tile_pool(name="work", bufs=3) as wpool,
            tc.tile_pool(name="small", bufs=4) as spool,
            tc.tile_pool(name="mlp", bufs=2, space=bass.MemorySpace.PSUM) as mlppool,
            tc.tile_pool(name="rps", bufs=1, space=bass.MemorySpace.PSUM) as rpool,
            tc.tile_pool(name="stat", bufs=1, space=bass.MemorySpace.PSUM) as statpool,
            tc.tile_pool(name="nout", bufs=2, space=bass.MemorySpace.PSUM) as npool,
        ):
            # ---- views onto the preamble-loaded constants ----
            outbuf = cpool.tile([4, NCHUNK * 64], dt)
            big3_sb = big3_t.ap()
            bigw_sb = bigw_t.ap()
            edata_sb = edata_t.ap().rearrange("p (a b) -> p a b", b=102)
            eps_sb = eps_t.ap()

            UOFF = 4 * EDGES_PER_CORE       # w1eff offset inside big3
            w3_sb = bigw_sb[:, 512:2048]
            ones_sb = bigw_sb[:, 2048:2049]   # holds 1/128
            xg0_sb = edata_sb[:, :, 0:16]
            xg1_sb = edata_sb[:, :, 16:64]
            b00_sb = edata_sb[:, :, 64:65]
            b01_sb = edata_sb[:, :, 65:68]
            b10_sb = edata_sb[:, :, 68:71]
            b11_sb = edata_sb[:, :, 71:98]
            wblk_sb = edata_sb[:, :, 98:102]



            for t in range(NTILE):
                e0 = t * 512
                # per-pair SBUF intermediates for this tile
                h2s = []
                r_all = spool.tile([128, 16], dt, tag="r_all")  # [P*4 + jj]
                for P in range(4):
                    psA = mlppool.tile([128, 512], dt, tag="ps")
                    psV = mlppool.tile([128, 512], dt, tag="ps")
                    h1 = wpool.tile([128, 512], dt, tag="h1")
                    sq = wpool.tile([128, 512], dt, tag="sq")
                    h2 = wpool.tile([128, 512], dt, tag=f"h2_{P}")
                    psVar = statpool.tile([128, 4], dt, tag="psVar")

                    # mm1: [3,128].T @ [3,512] -> [128,512]
                    nc.tensor.matmul(
                        psA[:],
                        big3_sb[:, UOFF + P * 128: UOFF + (P + 1) * 128],
                        big3_sb[:, P * EDGES_PER_CORE + e0:
                                P * EDGES_PER_CORE + e0 + 512],
                        start=True, stop=True,
                    )
                    nc.scalar.activation(h1[:], psA[:], AF.Relu)
                    # mm2 (w2 pre-centered): [128,128].T @ [128,512]
                    nc.tensor.matmul(
                        psV[:], bigw_sb[:, P * 128:(P + 1) * 128], h1[:],
                        start=True, stop=True,
                    )
                    # LN2 variance: square then per-chunk ones-matmul column sums
                    nc.scalar.activation(sq[:], psV[:], AF.Square)
                    for jj in range(4):
                        nc.tensor.matmul(
                            psVar[:, jj:jj + 1],
                            sq[:, jj * 128:(jj + 1) * 128],
                            ones_sb[:],
                            start=True, stop=True,
                        )
                    # r2 = rsqrt(var/128 + eps)  (edge-major [128,4])
                    sd = spool.tile([128, 4], dt, tag="sd")
                    nc.scalar.activation(sd[:], psVar[:], AF.Sqrt, bias=eps_sb[:])
                    if P == 0:
                        # fold basis00 per-edge scalar into pair00's r
                        rb = spool.tile([128, 4], dt, tag="rb")
                        nc.vector.reciprocal(rb[:], sd[:])
                        nc.vector.tensor_tensor(
                            r_all[:, 0:4], rb[:], b00_sb[:, t * 4:t * 4 + 4, 0],
                            op=ALU.mult,
                        )
                    else:
                        nc.vector.reciprocal(r_all[:, P * 4:(P + 1) * 4], sd[:])
                    # h2' = relu(v2c); g2 (>0) and r2 (>0) fold into w3 / k-mean
                    nc.scalar.activation(h2[:], psV[:], AF.Relu)
                    h2s.append(h2)

                # ---- per chunk: mm3 + einsum + masked k-mean ----
                for jj in range(4):
                    j = t * 4 + jj
                    psRa = rpool.tile([128, 512], dt, tag="psRa")   # R00|R01
                    psRb = rpool.tile([128, 1024], dt, tag="psRb")  # R11|R10
                    for P in range(2):
                        nc.tensor.matmul(
                            psRa[:, P * 256:(P + 1) * 256],
                            h2s[P][:, jj * 128:(jj + 1) * 128],
                            w3_sb[:, W3_OFF[P]:W3_OFF[P] + 256],
                            start=True, stop=True,
                        )
                    nc.tensor.matmul(
                        psRb[:, 0:512],
                        h2s[3][:, jj * 128:(jj + 1) * 128],
                        w3_sb[:, 768:1280],
                        start=True, stop=True,
                    )
                    nc.tensor.matmul(
                        psRb[:, 512:768],
                        h2s[3][:, jj * 128:(jj + 1) * 128],
                        w3_sb[:, 1280:1536],
                        start=True, stop=True,
                    )
                    nc.tensor.matmul(
                        psRb[:, 768:1024],
                        h2s[2][:, jj * 128:(jj + 1) * 128],
                        w3_sb[:, W3_OFF[2]:W3_OFF[2] + 256],
                        start=True, stop=True,
                    )

                    psN = npool.tile([4, 64], dt, tag="psN")

                    # --- pairs 00 & 01 share: s[e, P, o] = sum_i R[e,(P,o,i)]*xg0[e,i]
                    pbuf = wpool.tile([128, 512], dt, tag="pbuf")
                    s01 = spool.tile([128, 32], dt, tag="s01")
                    xg0j = xg0_sb[:, j, :]  # [128, 16]
                    nc.vector.tensor_tensor(
                        pbuf[:].rearrange("e (a i) -> e a i", i=16),
                        psRa[:, 0:512].rearrange("e (a i) -> e a i", i=16),
                        xg0j.unsqueeze(1).broadcast_to([128, 32, 16]),
                        op=ALU.mult,
                    )
                    nc.vector.tensor_reduce(
                        s01[:],
                        pbuf[:].rearrange("e (a i) -> e a i", i=16),
                        axis=AX.X, op=ALU.add,
                    )
                    # pair01 output: outP01[e, o, p] = s01[e, 16+o] * b01[e, p]
                    o01 = spool.tile([128, 48], dt, tag="o01")
                    nc.vector.tensor_tensor(
                        o01[:].rearrange("e (o p) -> e o p", p=3),
                        s01[:, 16:32].unsqueeze(2).broadcast_to([128, 16, 3]),
                        b01_sb[:, j, :].unsqueeze(1).broadcast_to([128, 16, 3]),
                        op=ALU.mult,
                    )

                    # --- pair10: t10[e,i] = sum_q xg1[e,i,q]*b10[e,q]
                    tb = spool.tile([128, 48], dt, tag="tb")
                    t10 = spool.tile([128, 16], dt, tag="t10")
                    o10 = spool.tile([128, 16], dt, tag="o10")
                    nc.vector.tensor_tensor(
                        tb[:].rearrange("e (i q) -> e i q", q=3),
                        xg1_sb[:, j, :].rearrange("e (i q) -> e i q", q=3),
                        b10_sb[:, j, :].unsqueeze(1).broadcast_to([128, 16, 3]),
                        op=ALU.mult,
                    )
                    nc.vector.tensor_reduce(
                        t10[:], tb[:].rearrange("e (i q) -> e i q", q=3),
                        axis=AX.X, op=ALU.add,
                    )
                    nc.vector.tensor_tensor(
                        pbuf[:, 0:256].rearrange("e (o i) -> e o i", i=16),
                        psRb[:, 768:1024].rearrange("e (o i) -> e o i", i=16),
                        t10[:].unsqueeze(1).broadcast_to([128, 16, 16]),
                        op=ALU.mult,
                    )
                    nc.vector.tensor_reduce(
                        o10[:], pbuf[:, 0:256].rearrange("e (o i) -> e o i", i=16),
                        axis=AX.X, op=ALU.add,
                    )

                    # --- pair11 ---
                    tbuf = wpool.tile([128, 432], dt, tag="tbuf")
                    t11 = spool.tile([128, 144], dt, tag="t11")
                    p11 = wpool.tile([128, 2304], dt, tag="p11")
                    o11 = spool.tile([128, 48], dt, tag="o11")
                    # T[e,(i,f,p)] = sum_q xg1[e,i,q] * b11[e,(p,q,f)]
                    # (split by p: HW APs allow at most 3 free dims)
                    tb5 = tbuf[:].rearrange("e (i f p q) -> e i f p q",
                                            f=3, p=3, q=3)
                    xg15 = (xg1_sb[:, j, :]
                            .rearrange("e (i q) -> e i q", q=3)
                            .unsqueeze(2)
                            .broadcast_to([128, 16, 3, 3]))
                    b115 = (b11_sb[:, j, :]
                            .rearrange("e (p q f) -> e p q f", q=3, f=3)
                            .transpose([0, 1, 3, 2]))  # -> [e, p, f, q]
                    for pp in range(3):
                        nc.vector.tensor_tensor(
                            tb5[:, :, :, pp, :],
                            xg15,
                            b115[:, pp, :, :].unsqueeze(1)
                            .broadcast_to([128, 16, 3, 3]),
                            op=ALU.mult,
                        )
                    nc.vector.tensor_reduce(
                        t11[:],
                        tbuf[:].rearrange("e (ifp q) -> e ifp q", q=3),
                        axis=AX.X, op=ALU.add,
                    )
                    # prod5[e, o,p,i,f] = R11[e,(o,i,f)] * T[e,(i,f,p)]
                    # (split by p: HW APs allow at most 3 free dims)
                    p115 = p11[:].rearrange("e (o p i f) -> e o p i f",
                                            o=16, p=3, i=16)
                    r115 = psRb[:, 0:768].rearrange("e (o i f) -> e o i f",
                                                    o=16, i=16)
                    t115 = t11[:].rearrange("e (i f p) -> e i f p", i=16, f=3)
                    for pp in range(3):
                        nc.vector.tensor_tensor(
                            p115[:, :, pp, :, :],
                            r115,
                            t115[:, :, :, pp].unsqueeze(1)
                            .broadcast_to([128, 16, 16, 3]),
                            op=ALU.mult,
                        )
                    nc.vector.tensor_reduce(
                        o11[:],
                        p11[:].rearrange("e (op i_f) -> e op i_f", i_f=48),
                        axis=AX.X, op=ALU.add,
                    )

                    # --- masked k-mean via block-diag PE matmuls ---
                    # wr_all[e, P, nn] = wblk[e, nn] * r_all[e, P*4+jj]
                    wr = spool.tile([128, 4, 4], dt, tag="wr")
                    nc.vector.tensor_tensor(
                        wr[:],
                        wblk_sb[:, j, :].unsqueeze(1).broadcast_to([128, 4, 4]),
                        r_all[:].rearrange("e (P c) -> e P c", c=4)[:, :, jj:jj + 1]
                        .broadcast_to([128, 4, 4]),
                        op=ALU.mult,
                    )
                    # do=0: pairs 00 (s01[:, :16]) and 10
                    nc.tensor.matmul(psN[:, 0:16], wr[:, 0, :], s01[:, 0:16],
                                     start=True, stop=False)
                    nc.tensor.matmul(psN[:, 0:16], wr[:, 2, :], o10[:],
                                     start=False, stop=True)
                    # do=1: pairs 01 and 11
                    nc.tensor.matmul(psN[:, 16:64], wr[:, 1, :], o01[:],
                                     start=True, stop=False)
                    nc.tensor.matmul(psN[:, 16:64], wr[:, 3, :], o11[:],
                                     start=False, stop=True)
                    # ACT copy: keeps this off the DVE critical path (Bacc's
                    # generate_event_semaphores legalizes any multi-wait)
                    nc.scalar.copy(outbuf[:, j * 64:(j + 1) * 64], psN[:])

            nc.sync.dma_start(out_d[:], outbuf[:])

    es.close()
    nc.compile()
    return nc


def _prep_core_inputs(x0, x1, rel_dist, basis, params, nidx, nmask):
    """Host-side folding + packing. Returns list of 8 in_maps."""
    f32 = np.float32
    # --- per-pair folded weights (shared across cores) ---
    w1eff = np.zeros((3, 4 * 128), f32)
    w2p = np.zeros((128, 4 * 128), f32)
    g2p = np.zeros((128, 4), f32)
    w3cat = np.zeros((128, 1536), f32)
    ones_col = np.full((128, 1), 1.0 / 128.0, f32)
    lnco = []  # (m20, m11, m02, mw, mb) per pair for u computation
    for P, pk in enumerate(PAIR_KEYS):
        rp = {k: np.asarray(v, f32) for k, v in params["rp"][pk].items()}
        w1 = rp["w1"][0]          # [128]
        b1 = rp["b1"]
        mw, mb = w1.mean(), b1.mean()
        wc, bc = w1 - mw, b1 - mb
        m20 = (wc * wc).mean()
        m11 = (wc * bc).mean()
        m02 = (bc * bc).mean()
        lnco.append((m20, m11, m02))
        w1eff[0, P * 128:(P + 1) * 128] = wc * rp["g1"]
        w1eff[1, P * 128:(P + 1) * 128] = bc * rp["g1"]
        w1eff[2, P * 128:(P + 1) * 128] = rp["be1"]
        w2p[:, P * 128:(P + 1) * 128] = rp["w2"] - rp["w2"].mean(1, keepdims=True)
        g2p[:, P] = rp["g2"]
        # w3 natural layout [128, o*i*f] == [c, (o,i,f)]; g2 (>0) folded in rows
        w3cat[:, W3_OFF[P]:W3_OFF[P] + W3_N[P]] = rp["g2"][:, None] * rp["w3"]
    # fast-path validity
    fast = True
    for pk in PAIR_KEYS:
        rp = params["rp"][pk]
        b2c = np.asarray(rp["b2"], f32)
        b2c = b2c - b2c.mean()
        if (np.abs(b2c).max() > 0 or np.abs(np.asarray(rp["be2"])).max() > 0
                or np.abs(np.asarray(rp["b3"])).max() > 0
                or np.asarray(rp["g2"]).min() <= 0):
            fast = False
    if not fast:
        return None

    nidx = np.asarray(nidx)
    nmask_f = np.asarray(nmask).astype(f32)
    denom = np.clip(nmask_f.sum(2), 1e-5, None)      # [B, N]
    wtil = nmask_f / denom[:, :, None]               # [B, N, K]

    in_maps = []
    for c in range(NCORES):
        b = c // 2
        n0 = (c % 2) * NODES_PER_CORE
        ns = slice(n0, n0 + NODES_PER_CORE)
        rd = np.asarray(rel_dist[b, ns], f32)        # [128, 32]
        idx = nidx[b, ns]                            # [128, 32]
        # edge order: e = n_local*32 + k ; chunk j = e//128 ; lane = e%128
        x = rd.reshape(-1)                           # [4096]
        u = np.zeros((3, 4 * EDGES_PER_CORE), f32)
        for P in range(4):
            m20, m11, m02 = lnco[P]
            r1 = 1.0 / np.sqrt(x * x * m20 + 2 * x * m11 + m02 + EPS)
            u[0, P * EDGES_PER_CORE:(P + 1) * EDGES_PER_CORE] = x * r1
            u[1, P * EDGES_PER_CORE:(P + 1) * EDGES_PER_CORE] = r1
            u[2, P * EDGES_PER_CORE:(P + 1) * EDGES_PER_CORE] = 1.0

        flat_idx = idx.reshape(-1)                   # [4096] node ids
        xg0 = np.asarray(x0[b], f32)[flat_idx, :, 0]          # [4096, 16]
        xg1 = np.asarray(x1[b], f32)[flat_idx].reshape(-1, 48)  # [4096, 16*3]
        bas = {pk: np.asarray(basis[pk][b, ns], f32).reshape(EDGES_PER_CORE, -1)
               for pk in PAIR_KEYS}
        wt = wtil[b, ns].reshape(-1)                 # [4096]
        # block-diag k-mean weights: wblk[e, nn] = wt[e] if (e%128)//32==nn
        wblk = np.zeros((EDGES_PER_CORE, 4), f32)
        lane = np.arange(EDGES_PER_CORE) % 128
        wblk[np.arange(EDGES_PER_CORE), lane // 32] = wt

        def pack(a):  # [4096, F] -> [128, NCHUNK*F] with [lane, chunk, F]
            F = a.shape[1]
            return np.ascontiguousarray(
                a.reshape(NCHUNK, 128, F).transpose(1, 0, 2).reshape(128, NCHUNK * F)
            )

        edata = np.concatenate(
            [xg0, xg1, bas["0,0"], bas["0,1"], bas["1,0"], bas["1,1"], wblk],
            axis=1)  # [4096, 102]
        in_maps.append({
            "big3": np.concatenate([u, w1eff], axis=1),
            "bigw": np.concatenate([w2p, w3cat, ones_col], axis=1),
            "edata": pack(edata),
        })
    return in_maps


def kernel(x0, x1, rel_dist, basis00, basis01, basis10, basis11, params,
           neighbor_indices, neighbor_masks):
    x0 = np.asarray(x0, np.float32)
    x1 = np.asarray(x1, np.float32)
    basis = {"0,0": np.asarray(basis00), "0,1": np.asarray(basis01),
             "1,0": np.asarray(basis10), "1,1": np.asarray(basis11)}

    in_maps = _prep_core_inputs(x0, x1, rel_dist, basis, params,
                                neighbor_indices, neighbor_masks)
    if in_maps is None:  # general-params fallback (never taken for this module)
        return _host_forward(x0, x1, np.asarray(rel_dist, np.float32), basis,
                             params, np.asarray(neighbor_indices),
                             np.asarray(neighbor_masks))

    if "nc" not in _CACHE:
        _CACHE["nc"] = _build_bass()
    nc = _CACHE["nc"]

    from concourse import bass_utils
    res = bass_utils.run_bass_kernel_spmd(nc, in_maps, core_ids=list(range(NCORES)))
    outs = res.results

    # reassemble: outd [4, NCHUNK*64] -> per core nodes
    out0 = np.zeros((B, N, 16), np.float32)
    out1 = np.zeros((B, N, 48), np.float32)
    for c in range(NCORES):
        od = outs[c]["outd"].reshape(4, NCHUNK, 64)   # [nn, chunk, 64]
        b = c // 2
        n0 = (c % 2) * NODES_PER_CORE
        # node = n0 + chunk*4 + nn
        o = od.transpose(1, 0, 2)                     # [chunk, nn, 64]
        o = o.reshape(NODES_PER_CORE, 64)
        out0[b, n0:n0 + NODES_PER_CORE] = o[:, 0:16]
        out1[b, n0:n0 + NODES_PER_CORE] = o[:, 16:64]

    out0 = out0.reshape(B, N, 16, 1)
    out1 = out1.reshape(B, N, 16, 3)
    # self-interaction (host, tiny)
    w_si0 = np.asarray(params["w_si0"], np.float32)
    w_si1 = np.asarray(params["w_si1"], np.float32)
    out0 = out0 + np.einsum("bndm,de->bnem", x0, w_si0)
    out1 = out1 + np.einsum("bndm,de->bnem", x1, w_si1)
    return out0.astype(np.float32), out1.astype(np.float32)


# revision 54
# speedup vs baseline: 1.3245x; 1.2923x over previous
"""ConvSE3 (SE(3)-equivariant graph conv) Trainium2 kernel, 8-core SPMD.

Strategy:
  - Shard (b, n): 4 batches x 256 nodes = 1024 rows -> 8 cores x 128 nodes.
    Each core: 128 nodes x 32 neighbors = 4096 edges (32 chunks of 128).
  - Host: gathers neighbor features, folds LayerNorm-1 analytically into a
    K=3 matmul (input is a per-edge scalar), centers w2/b2 so LN2's mean
    subtraction is free, computes masked-mean weights, packs per-core arrays.
  - Device per pair (di,do) of degree pairs:
      mm1 (K=3) -> relu -> mm2 (K=128) -> LN2 var via PE ones-matmuls ->
      rsqrt (ACT) -> relu(v2c*g2) (ACT, r2>0 folds past relu) -> mm3 -> R in
      PSUM -> per-edge einsum on DVE (edge-major, broadcast APs + segmented
      tensor_reduce) -> masked k-mean as PE matmul with block-diagonal
      per-edge weights (r2 and basis00 folded into those weights).
  - Host: adds self-interaction (tiny einsum) and reassembles full outputs.

Assumes be2 == 0, b3 == 0 and b2 - mean(b2) == 0 on the fast path (true for
this module's init); otherwise falls back to an exact host computation.
"""

import sys
import numpy as np

sys.path.insert(0, "/opt/trn_rl_repo")

B, N, K, M = 4, 256, 32, 16
MID = 128
EPS = 1e-5
NCORES = 8
NODES_PER_CORE = N // 2          # 128
EDGES_PER_CORE = NODES_PER_CORE * K   # 4096
NCHUNK = EDGES_PER_CORE // 128   # 32
NTILE = EDGES_PER_CORE // 512    # 8

# pair order: (di, do) with f = min(2di+1, 2do+1), p = 2do+1, q = 2di+1
PAIRS = [("0", "0"), ("0", "1"), ("1", "0"), ("1", "1")]
PAIR_KEYS = ["0,0", "0,1", "1,0", "1,1"]
PF = [1, 1, 1, 3]                 # f per pair
PP = [1, 3, 1, 3]                 # p per pair
PQ = [1, 1, 3, 3]                 # q per pair
W3_OFF = [0, 256, 512, 768]       # offsets in concatenated w3 [128, 1536]
W3_N = [256, 256, 256, 768]

_CACHE = {}
LINEARIZE = False


def _host_forward(x0, x1, rel_dist, basis, params, neighbor_indices, neighbor_masks):
    """Exact numpy mirror of the jax reference (fallback path)."""
    def ln(x, g, b):
        mu = x.mean(-1, keepdims=True)
        var = ((x - mu) ** 2).mean(-1, keepdims=True)
        return (x - mu) / np.sqrt(var + EPS) * g + b

    def radial(feat, p, o, i, f):
        h = np.maximum(ln(feat @ p["w1"] + p["b1"], p["g1"], p["be1"]), 0)
        h = np.maximum(ln(h @ p["w2"] + p["b2"], p["g2"], p["be2"]), 0)
        y = h @ p["w3"] + p["b3"]
        return y.reshape(y.shape[:-1] + (o, i, f))

    ORD = {"0": 1, "1": 3}
    b, n, k = neighbor_indices.shape
    m = x0.shape[2]
    feat = rel_dist[..., None]
    bidx = np.arange(b)[:, None, None]
    xg = {"0": x0[bidx, neighbor_indices], "1": x1[bidx, neighbor_indices]}
    outs = {}
    for do in ("0", "1"):
        acc = 0.0
        for di in ("0", "1"):
            pk = di + "," + do
            f = min(ORD[di], ORD[do])
            R = radial(feat, params["rp"][pk], m, m, f)
            acc = acc + np.einsum("bnkoif,bnkpqf,bnkiq->bnkop", R, basis[pk], xg[di])
        w = neighbor_masks[..., None, None].astype(x0.dtype)
        outs[do] = (acc * w).sum(2) / np.clip(w.sum(2), 1e-5, None)
    out0 = outs["0"] + np.einsum("bndm,de->bnem", x0, params["w_si0"])
    out1 = outs["1"] + np.einsum("bndm,de->bnem", x1, params["w_si1"])
    return out0.astype(np.float32), out1.astype(np.float32)


def _build_bass():
    """Build the 8-core SPMD Bass program (same program, per-core inputs)."""
    import concourse.bass as bass
    import concourse.mybir as mybir
    from concourse import bacc, tile

    dt = mybir.dt.float32
    AF = mybir.ActivationFunctionType
    ALU = mybir.AluOpType
    AX = mybir.AxisListType

    # Bacc (not raw Bass): its compile() runs move_matmul_waits_to_ldweights
    # and generate_event_semaphores, which legalize multi-wait instructions
    # for the TRN2 sync-struct limits.
    nc = bacc.Bacc(None, target_bir_lowering=False, debug=False)

    # ---- DRAM I/O (per-core shapes) ----
    # big3 = u [3, 4*4096] ++ w1eff [3, 512]  (one DMA -> one queue sem)
    big3_d = nc.dram_tensor("big3", [3, 4 * EDGES_PER_CORE + 512], dt,
                            kind="ExternalInput")
    # bigw = w2p [128,512] ++ w3cat(g2-folded) [128,1536] ++ ones/128 [128,1]
    bigw_d = nc.dram_tensor("bigw", [128, 2049], dt, kind="ExternalInput")
    # edata: per-edge chunk-major data, 102 f32/edge:
    #   xg0 0:16 | xg1 16:64 | b00 64:65 | b01 65:68 | b10 68:71 |
    #   b11 71:98 | wblk 98:102
    edata_d = nc.dram_tensor("edata", [128, NCHUNK * 102], dt,
                             kind="ExternalInput")
    out_d = nc.dram_tensor("outd", [4, NCHUNK * 64], dt, kind="ExternalOutput")

    # ---- preamble (raw bass, before Tile): load all constants, then an
    # all-engine barrier. Tile then sees no DMA producers, so compute
    # instructions never need DMA-queue waits (PE matmul allows only one
    # wait; multi-wait instructions fail walrus codegen).
    big3_t = nc.alloc_sbuf_tensor("big3_sb", [3, 4 * EDGES_PER_CORE + 512], dt)
    bigw_t = nc.alloc_sbuf_tensor("bigw_sb", [128, 2049], dt)
    edata_t = nc.alloc_sbuf_tensor("edata_sb", [128, NCHUNK * 102], dt)
    eps_t = nc.alloc_sbuf_tensor("eps_sb", [128, 1], dt)
    from contextlib import ExitStack
    es = ExitStack()
    ldsem = es.enter_context(nc.semaphore())
    nc.sync.dma_start(big3_t.ap(), big3_d[:]).then_inc(ldsem, 16)
    nc.sync.dma_start(bigw_t.ap(), bigw_d[:]).then_inc(ldsem, 16)
    nc.sync.dma_start(edata_t.ap(), edata_d[:]).then_inc(ldsem, 16)
    nc.gpsimd.memset(eps_t.ap(), EPS)
    for eng in (nc.tensor, nc.vector, nc.scalar, nc.gpsimd, nc.sync):
        eng.wait_ge(ldsem, 48)
    nc.all_engine_barrier()

    with tile.TileContext(nc, linearize=LINEARIZE) as tc:
        with (
            tc.tile_pool(name="const", bufs=1) as cpool,
            tc.tile_pool(name="work", bufs=3) as wpool,
            tc.tile_pool(name="small", bufs=4) as spool,
            tc.tile_pool(name="mlp", bufs=2, space=bass.MemorySpace.PSUM) as mlppool,
            tc.tile_pool(name="rps", bufs=1, space=bass.MemorySpace.PSUM) as rpool,
            tc.tile_pool(name="stat", bufs=1, space=bass.MemorySpace.PSUM) as statpool,
            tc.tile_pool(name="nout", bufs=2, space=bass.MemorySpace.PSUM) as npool,
        ):
            # ---- views onto the preamble-loaded constants ----
            outbuf = cpool.tile([4, NCHUNK * 64], dt)
            big3_sb = big3_t.ap()
            bigw_sb = bigw_t.ap()
            edata_sb = edata_t.ap().rearrange("p (a b) -> p a b", b=102)
            eps_sb = eps_t.ap()

            UOFF = 4 * EDGES_PER_CORE       # w1eff offset inside big3
            w3_sb = bigw_sb[:, 512:2048]
            ones_sb = bigw_sb[:, 2048:2049]   # holds 1/128
            xg0_sb = edata_sb[:, :, 0:16]
            xg1_sb = edata_sb[:, :, 16:64]
            b00_sb = edata_sb[:, :, 64:65]
            b01_sb = edata_sb[:, :, 65:68]
            b10_sb = edata_sb[:, :, 68:71]
            b11_sb = edata_sb[:, :, 71:98]
            wblk_sb = edata_sb[:, :, 98:102]



            for t in range(NTILE):
                e0 = t * 512
                # per-pair SBUF intermediates for this tile
                h2s = []
                r_all = spool.tile([128, 16], dt, tag="r_all")  # [P*4 + jj]
                for P in range(4):
                    psA = mlppool.tile([128, 512], dt, tag="ps")
                    psV = mlppool.tile([128, 512], dt, tag="ps")
                    h1 = wpool.tile([128, 512], dt, tag="h1")
                    sq = wpool.tile([128, 512], dt, tag="sq")
                    h2 = wpool.tile([128, 512], dt, tag=f"h2_{P}")
                    psVar = statpool.tile([128, 4], dt, tag="psVar")

                    # mm1: [3,128].T @ [3,512] -> [128,512]
                    nc.tensor.matmul(
                        psA[:],
                        big3_sb[:, UOFF + P * 128: UOFF + (P + 1) * 128],
                        big3_sb[:, P * EDGES_PER_CORE + e0:
                                P * EDGES_PER_CORE + e0 + 512],
                        start=True, stop=True,
                    )
                    nc.scalar.activation(h1[:], psA[:], AF.Relu)
                    # mm2 (w2 pre-centered): [128,128].T @ [128,512]
                    nc.tensor.matmul(
                        psV[:], bigw_sb[:, P * 128:(P + 1) * 128], h1[:],
                        start=True, stop=True,
                    )
                    # LN2 variance: square then per-chunk ones-matmul column sums
                    nc.scalar.activation(sq[:], psV[:], AF.Square)
                    for jj in range(4):
                        nc.tensor.matmul(
                            psVar[:, jj:jj + 1],
                            sq[:, jj * 128:(jj + 1) * 128],
                            ones_sb[:],
                            start=True, stop=True,
                        )
                    # r2 = rsqrt(var/128 + eps)  (edge-major [128,4])
                    sd = spool.tile([128, 4], dt, tag="sd")
                    nc.scalar.activation(sd[:], psVar[:], AF.Sqrt, bias=eps_sb[:])
                    if P == 0:
                        # fold basis00 per-edge scalar into pair00's r
                        rb = spool.tile([128, 4], dt, tag="rb")
                        nc.vector.reciprocal(rb[:], sd[:])
                        nc.vector.tensor_tensor(
                            r_all[:, 0:4], rb[:], b00_sb[:, t * 4:t * 4 + 4, 0],
                            op=ALU.mult,
                        )
                    else:
                        nc.vector.reciprocal(r_all[:, P * 4:(P + 1) * 4], sd[:])
                    # h2' = relu(v2c); g2 (>0) and r2 (>0) fold into w3 / k-mean
                    nc.scalar.activation(h2[:], psV[:], AF.Relu)
                    h2s.append(h2)

                # ---- per chunk: mm3 + einsum + masked k-mean ----
                for jj in range(4):
                    j = t * 4 + jj
                    psRa = rpool.tile([128, 512], dt, tag="psRa")   # R00|R01
                    psRb = rpool.tile([128, 1024], dt, tag="psRb")  # R11|R10
                    for P in range(2):
                        nc.tensor.matmul(
                            psRa[:, P * 256:(P + 1) * 256],
                            h2s[P][:, jj * 128:(jj + 1) * 128],
                            w3_sb[:, W3_OFF[P]:W3_OFF[P] + 256],
                            start=True, stop=True,
                        )
                    nc.tensor.matmul(
                        psRb[:, 0:512],
                        h2s[3][:, jj * 128:(jj + 1) * 128],
                        w3_sb[:, 768:1280],
                        start=True, stop=True,
                    )
                    nc.tensor.matmul(
                        psRb[:, 512:768],
                        h2s[3][:, jj * 128:(jj + 1) * 128],
                        w3_sb[:, 1280:1536],
                        start=True, stop=True,
                    )
                    nc.tensor.matmul(
                        psRb[:, 768:1024],
                        h2s[2][:, jj * 128:(jj + 1) * 128],
                        w3_sb[:, W3_OFF[2]:W3_OFF[2] + 256],
                        start=True, stop=True,
                    )

                    psN = npool.tile([4, 64], dt, tag="psN")

                    # stage f=1 R blocks PSUM->SBUF on ACT so GPSIMD can run
                    # their product passes (GPSIMD has no PSUM port); also
                    # releases psRa earlier for the next chunk's mm3.
                    ra = wpool.tile([128, 512], dt, tag="ra")
                    r10 = wpool.tile([128, 256], dt, tag="r10")
                    nc.scalar.copy(ra[:], psRa[:, 0:512])
                    nc.scalar.copy(r10[:], psRb[:, 768:1024])

                    # --- pairs 00 & 01 share: s[e, P, o] = sum_i R[e,(P,o,i)]*xg0[e,i]
                    pbuf = wpool.tile([128, 512], dt, tag="pbuf")
                    pbuf2 = wpool.tile([128, 256], dt, tag="pbuf2")
                    s01 = spool.tile([128, 32], dt, tag="s01")
                    xg0j = xg0_sb[:, j, :]  # [128, 16]
                    nc.gpsimd.tensor_tensor(
                        pbuf[:].rearrange("e (a i) -> e a i", i=16),
                        ra[:].rearrange("e (a i) -> e a i", i=16),
                        xg0j.unsqueeze(1).broadcast_to([128, 32, 16]),
                        op=ALU.mult,
                    )
                    nc.vector.tensor_reduce(
                        s01[:],
                        pbuf[:].rearrange("e (a i) -> e a i", i=16),
                        axis=AX.X, op=ALU.add,
                    )
                    # pair01 output: outP01[e, o, p] = s01[e, 16+o] * b01[e, p]
                    o01 = spool.tile([128, 48], dt, tag="o01")
                    nc.gpsimd.tensor_tensor(
                        o01[:].rearrange("e (o p) -> e o p", p=3),
                        s01[:, 16:32].unsqueeze(2).broadcast_to([128, 16, 3]),
                        b01_sb[:, j, :].unsqueeze(1).broadcast_to([128, 16, 3]),
                        op=ALU.mult,
                    )

                    # --- pair10: t10[e,i] = sum_q xg1[e,i,q]*b10[e,q]
                    tb = spool.tile([128, 48], dt, tag="tb")
                    t10 = spool.tile([128, 16], dt, tag="t10")
                    o10 = spool.tile([128, 16], dt, tag="o10")
                    nc.gpsimd.tensor_tensor(
                        tb[:].rearrange("e (i q) -> e i q", q=3),
                        xg1_sb[:, j, :].rearrange("e (i q) -> e i q", q=3),
                        b10_sb[:, j, :].unsqueeze(1).broadcast_to([128, 16, 3]),
                        op=ALU.mult,
                    )
                    nc.vector.tensor_reduce(
                        t10[:], tb[:].rearrange("e (i q) -> e i q", q=3),
                        axis=AX.X, op=ALU.add,
                    )
                    nc.gpsimd.tensor_tensor(
                        pbuf2[:].rearrange("e (o i) -> e o i", i=16),
                        r10[:].rearrange("e (o i) -> e o i", i=16),
                        t10[:].unsqueeze(1).broadcast_to([128, 16, 16]),
                        op=ALU.mult,
                    )
                    nc.vector.tensor_reduce(
                        o10[:], pbuf2[:].rearrange("e (o i) -> e o i", i=16),
                        axis=AX.X, op=ALU.add,
                    )

                    # --- pair11 ---
                    tbuf = wpool.tile([128, 432], dt, tag="tbuf")
                    t11 = spool.tile([128, 144], dt, tag="t11")
                    p11 = wpool.tile([128, 2304], dt, tag="p11")
                    o11 = spool.tile([128, 48], dt, tag="o11")
                    # T[e,(i,f,p)] = sum_q xg1[e,i,q] * b11[e,(p,q,f)]
                    # (split by p: HW APs allow at most 3 free dims)
                    tb5 = tbuf[:].rearrange("e (i f p q) -> e i f p q",
                                            f=3, p=3, q=3)
                    xg15 = (xg1_sb[:, j, :]
                            .rearrange("e (i q) -> e i q", q=3)
                            .unsqueeze(2)
                            .broadcast_to([128, 16, 3, 3]))
                    b115 = (b11_sb[:, j, :]
                            .rearrange("e (p q f) -> e p q f", q=3, f=3)
                            .transpose([0, 1, 3, 2]))  # -> [e, p, f, q]
                    for pp in range(3):
                        nc.vector.tensor_tensor(
                            tb5[:, :, :, pp, :],
                            xg15,
                            b115[:, pp, :, :].unsqueeze(1)
                            .broadcast_to([128, 16, 3, 3]),
                            op=ALU.mult,
                        )
                    nc.vector.tensor_reduce(
                        t11[:],
                        tbuf[:].rearrange("e (ifp q) -> e ifp q", q=3),
                        axis=AX.X, op=ALU.add,
                    )
                    # prod5[e, o,p,i,f] = R11[e,(o,i,f)] * T[e,(i,f,p)]
                    # (split by p: HW APs allow at most 3 free dims)
                    p115 = p11[:].rearrange("e (o p i f) -> e o p i f",
                                            o=16, p=3, i=16)
                    r115 = psRb[:, 0:768].rearrange("e (o i f) -> e o i f",
                                                    o=16, i=16)
                    t115 = t11[:].rearrange("e (i f p) -> e i f p", i=16, f=3)
                    for pp in range(3):
                        nc.vector.tensor_tensor(
                            p115[:, :, pp, :, :],
                            r115,
                            t115[:, :, :, pp].unsqueeze(1)
                            .broadcast_to([128, 16, 16, 3]),
                            op=ALU.mult,
                        )
                    nc.vector.tensor_reduce(
                        o11[:],
                        p11[:].rearrange("e (op i_f) -> e op i_f", i_f=48),
                        axis=AX.X, op=ALU.add,
                    )

                    # --- masked k-mean via block-diag PE matmuls ---
                    # wr_all[e, P, nn] = wblk[e, nn] * r_all[e, P*4+jj]
                    wr = spool.tile([128, 4, 4], dt, tag="wr")
                    nc.vector.tensor_tensor(
                        wr[:],
                        wblk_sb[:, j, :].unsqueeze(1).broadcast_to([128, 4, 4]),
                        r_all[:].rearrange("e (P c) -> e P c", c=4)[:, :, jj:jj + 1]
                        .broadcast_to([128, 4, 4]),
                        op=ALU.mult,
                    )
                    # do=0: pairs 00 (s01[:, :16]) and 10
                    nc.tensor.matmul(psN[:, 0:16], wr[:, 0, :], s01[:, 0:16],
                                     start=True, stop=False)
                    nc.tensor.matmul(psN[:, 0:16], wr[:, 2, :], o10[:],
                                     start=False, stop=True)
                    # do=1: pairs 01 and 11
                    nc.tensor.matmul(psN[:, 16:64], wr[:, 1, :], o01[:],
                                     start=True, stop=False)
                    nc.tensor.matmul(psN[:, 16:64], wr[:, 3, :], o11[:],
                                     start=False, stop=True)
                    # ACT copy: keeps this off the DVE critical path (Bacc's
                    # generate_event_semaphores legalizes any multi-wait)
                    nc.scalar.copy(outbuf[:, j * 64:(j + 1) * 64], psN[:])

            nc.sync.dma_start(out_d[:], outbuf[:])

    es.close()
    nc.compile()
    return nc


def _prep_core_inputs(x0, x1, rel_dist, basis, params, nidx, nmask):
    """Host-side folding + packing. Returns list of 8 in_maps."""
    f32 = np.float32
    # --- per-pair folded weights (shared across cores) ---
    w1eff = np.zeros((3, 4 * 128), f32)
    w2p = np.zeros((128, 4 * 128), f32)
    g2p = np.zeros((128, 4), f32)
    w3cat = np.zeros((128, 1536), f32)
    ones_col = np.full((128, 1), 1.0 / 128.0, f32)
    lnco = []  # (m20, m11, m02, mw, mb) per pair for u computation
    for P, pk in enumerate(PAIR_KEYS):
        rp = {k: np.asarray(v, f32) for k, v in params["rp"][pk].items()}
        w1 = rp["w1"][0]          # [128]
        b1 = rp["b1"]
        mw, mb = w1.mean(), b1.mean()
        wc, bc = w1 - mw, b1 - mb
        m20 = (wc * wc).mean()
        m11 = (wc * bc).mean()
        m02 = (bc * bc).mean()
        lnco.append((m20, m11, m02))
        w1eff[0, P * 128:(P + 1) * 128] = wc * rp["g1"]
        w1eff[1, P * 128:(P + 1) * 128] = bc * rp["g1"]
        w1eff[2, P * 128:(P + 1) * 128] = rp["be1"]
        w2p[:, P * 128:(P + 1) * 128] = rp["w2"] - rp["w2"].mean(1, keepdims=True)
        g2p[:, P] = rp["g2"]
        # w3 natural layout [128, o*i*f] == [c, (o,i,f)]; g2 (>0) folded in rows
        w3cat[:, W3_OFF[P]:W3_OFF[P] + W3_N[P]] = rp["g2"][:, None] * rp["w3"]
    # fast-path validity
    fast = True
    for pk in PAIR_KEYS:
        rp = params["rp"][pk]
        b2c = np.asarray(rp["b2"], f32)
        b2c = b2c - b2c.mean()
        if (np.abs(b2c).max() > 0 or np.abs(np.asarray(rp["be2"])).max() > 0
                or np.abs(np.asarray(rp["b3"])).max() > 0
                or np.asarray(rp["g2"]).min() <= 0):
            fast = False
    if not fast:
        return None

    nidx = np.asarray(nidx)
    nmask_f = np.asarray(nmask).astype(f32)
    denom = np.clip(nmask_f.sum(2), 1e-5, None)      # [B, N]
    wtil = nmask_f / denom[:, :, None]               # [B, N, K]

    in_maps = []
    for c in range(NCORES):
        b = c // 2
        n0 = (c % 2) * NODES_PER_CORE
        ns = slice(n0, n0 + NODES_PER_CORE)
        rd = np.asarray(rel_dist[b, ns], f32)        # [128, 32]
        idx = nidx[b, ns]                            # [128, 32]
        # edge order: e = n_local*32 + k ; chunk j = e//128 ; lane = e%128
        x = rd.reshape(-1)                           # [4096]
        u = np.zeros((3, 4 * EDGES_PER_CORE), f32)
        for P in range(4):
            m20, m11, m02 = lnco[P]
            r1 = 1.0 / np.sqrt(x * x * m20 + 2 * x * m11 + m02 + EPS)
            u[0, P * EDGES_PER_CORE:(P + 1) * EDGES_PER_CORE] = x * r1
            u[1, P * EDGES_PER_CORE:(P + 1) * EDGES_PER_CORE] = r1
            u[2, P * EDGES_PER_CORE:(P + 1) * EDGES_PER_CORE] = 1.0

        flat_idx = idx.reshape(-1)                   # [4096] node ids
        xg0 = np.asarray(x0[b], f32)[flat_idx, :, 0]          # [4096, 16]
        xg1 = np.asarray(x1[b], f32)[flat_idx].reshape(-1, 48)  # [4096, 16*3]
        bas = {pk: np.asarray(basis[pk][b, ns], f32).reshape(EDGES_PER_CORE, -1)
               for pk in PAIR_KEYS}
        wt = wtil[b, ns].reshape(-1)                 # [4096]
        # block-diag k-mean weights: wblk[e, nn] = wt[e] if (e%128)//32==nn
        wblk = np.zeros((EDGES_PER_CORE, 4), f32)
        lane = np.arange(EDGES_PER_CORE) % 128
        wblk[np.arange(EDGES_PER_CORE), lane // 32] = wt

        def pack(a):  # [4096, F] -> [128, NCHUNK*F] with [lane, chunk, F]
            F = a.shape[1]
            return np.ascontiguousarray(
                a.reshape(NCHUNK, 128, F).transpose(1, 0, 2).reshape(128, NCHUNK * F)
            )

        edata = np.concatenate(
            [xg0, xg1, bas["0,0"], bas["0,1"], bas["1,0"], bas["1,1"], wblk],
            axis=1)  # [4096, 102]
        in_maps.append({
            "big3": np.concatenate([u, w1eff], axis=1),
            "bigw": np.concatenate([w2p, w3cat, ones_col], axis=1),
            "edata": pack(edata),
        })
    return in_maps


def kernel(x0, x1, rel_dist, basis00, basis01, basis10, basis11, params,
           neighbor_indices, neighbor_masks):
    x0 = np.asarray(x0, np.float32)
    x1 = np.asarray(x1, np.float32)
    basis = {"0,0": np.asarray(basis00), "0,1": np.asarray(basis01),
             "1,0": np.asarray(basis10), "1,1": np.asarray(basis11)}

    in_maps = _prep_core_inputs(x0, x1, rel_dist, basis, params,
                                neighbor_indices, neighbor_masks)
    if in_maps is None:  # general-params fallback (never taken for this module)
        return _host_forward(x0, x1, np.asarray(rel_dist, np.float32), basis,
                             params, np.asarray(neighbor_indices),
                             np.asarray(neighbor_masks))

    if "nc" not in _CACHE:
        _CACHE["nc"] = _build_bass()
    nc = _CACHE["nc"]

    from concourse import bass_utils
    res = bass_utils.run_bass_kernel_spmd(nc, in_maps, core_ids=list(range(NCORES)))
    outs = res.results

    # reassemble: outd [4, NCHUNK*64] -> per core nodes
    out0 = np.zeros((B, N, 16), np.float32)
    out1 = np.zeros((B, N, 48), np.float32)
    for c in range(NCORES):
        od = outs[c]["outd"].reshape(4, NCHUNK, 64)   # [nn, chunk, 64]
        b = c // 2
        n0 = (c % 2) * NODES_PER_CORE
        # node = n0 + chunk*4 + nn
        o = od.transpose(1, 0, 2)                     # [chunk, nn, 64]
        o = o.reshape(NODES_PER_CORE, 64)
        out0[b, n0:n0 + NODES_PER_CORE] = o[:, 0:16]
        out1[b, n0:n0 + NODES_PER_CORE] = o[:, 16:64]

    out0 = out0.reshape(B, N, 16, 1)
    out1 = out1.reshape(B, N, 16, 3)
    # self-interaction (host, tiny)
    w_si0 = np.asarray(params["w_si0"], np.float32)
    w_si1 = np.asarray(params["w_si1"], np.float32)
    out0 = out0 + np.einsum("bndm,de->bnem", x0, w_si0)
    out1 = out1 + np.einsum("bndm,de->bnem", x1, w_si1)
    return out0.astype(np.float32), out1.astype(np.float32)


# revision 55
# speedup vs baseline: 1.4337x; 1.0824x over previous
"""ConvSE3 (SE(3)-equivariant graph conv) Trainium2 kernel, 8-core SPMD.

Strategy:
  - Shard (b, n): 4 batches x 256 nodes = 1024 rows -> 8 cores x 128 nodes.
    Each core: 128 nodes x 32 neighbors = 4096 edges (32 chunks of 128).
  - Host: gathers neighbor features, folds LayerNorm-1 analytically into a
    K=3 matmul (input is a per-edge scalar), centers w2/b2 so LN2's mean
    subtraction is free, computes masked-mean weights, packs per-core arrays.
  - Device per pair (di,do) of degree pairs:
      mm1 (K=3) -> relu -> mm2 (K=128) -> LN2 var via PE ones-matmuls ->
      rsqrt (ACT) -> relu(v2c*g2) (ACT, r2>0 folds past relu) -> mm3 -> R in
      PSUM -> per-edge einsum on DVE (edge-major, broadcast APs + segmented
      tensor_reduce) -> masked k-mean as PE matmul with block-diagonal
      per-edge weights (r2 and basis00 folded into those weights).
  - Host: adds self-interaction (tiny einsum) and reassembles full outputs.

Assumes be2 == 0, b3 == 0 and b2 - mean(b2) == 0 on the fast path (true for
this module's init); otherwise falls back to an exact host computation.
"""

import sys
import numpy as np

sys.path.insert(0, "/opt/trn_rl_repo")

B, N, K, M = 4, 256, 32, 16
MID = 128
EPS = 1e-5
NCORES = 8
NODES_PER_CORE = N // 2          # 128
EDGES_PER_CORE = NODES_PER_CORE * K   # 4096
NCHUNK = EDGES_PER_CORE // 128   # 32
NTILE = EDGES_PER_CORE // 512    # 8

# pair order: (di, do) with f = min(2di+1, 2do+1), p = 2do+1, q = 2di+1
PAIRS = [("0", "0"), ("0", "1"), ("1", "0"), ("1", "1")]
PAIR_KEYS = ["0,0", "0,1", "1,0", "1,1"]
PF = [1, 1, 1, 3]                 # f per pair
PP = [1, 3, 1, 3]                 # p per pair
PQ = [1, 1, 3, 3]                 # q per pair
W3_OFF = [0, 256, 512, 768]       # offsets in concatenated w3 [128, 1536]
W3_N = [256, 256, 256, 768]

_CACHE = {}
LINEARIZE = False


def _host_forward(x0, x1, rel_dist, basis, params, neighbor_indices, neighbor_masks):
    """Exact numpy mirror of the jax reference (fallback path)."""
    def ln(x, g, b):
        mu = x.mean(-1, keepdims=True)
        var = ((x - mu) ** 2).mean(-1, keepdims=True)
        return (x - mu) / np.sqrt(var + EPS) * g + b

    def radial(feat, p, o, i, f):
        h = np.maximum(ln(feat @ p["w1"] + p["b1"], p["g1"], p["be1"]), 0)
        h = np.maximum(ln(h @ p["w2"] + p["b2"], p["g2"], p["be2"]), 0)
        y = h @ p["w3"] + p["b3"]
        return y.reshape(y.shape[:-1] + (o, i, f))

    ORD = {"0": 1, "1": 3}
    b, n, k = neighbor_indices.shape
    m = x0.shape[2]
    feat = rel_dist[..., None]
    bidx = np.arange(b)[:, None, None]
    xg = {"0": x0[bidx, neighbor_indices], "1": x1[bidx, neighbor_indices]}
    outs = {}
    for do in ("0", "1"):
        acc = 0.0
        for di in ("0", "1"):
            pk = di + "," + do
            f = min(ORD[di], ORD[do])
            R = radial(feat, params["rp"][pk], m, m, f)
            acc = acc + np.einsum("bnkoif,bnkpqf,bnkiq->bnkop", R, basis[pk], xg[di])
        w = neighbor_masks[..., None, None].astype(x0.dtype)
        outs[do] = (acc * w).sum(2) / np.clip(w.sum(2), 1e-5, None)
    out0 = outs["0"] + np.einsum("bndm,de->bnem", x0, params["w_si0"])
    out1 = outs["1"] + np.einsum("bndm,de->bnem", x1, params["w_si1"])
    return out0.astype(np.float32), out1.astype(np.float32)


def _build_bass():
    """Build the 8-core SPMD Bass program (same program, per-core inputs)."""
    import concourse.bass as bass
    import concourse.mybir as mybir
    from concourse import bacc, tile

    dt = mybir.dt.float32
    AF = mybir.ActivationFunctionType
    ALU = mybir.AluOpType
    AX = mybir.AxisListType

    # Bacc (not raw Bass): its compile() runs move_matmul_waits_to_ldweights
    # and generate_event_semaphores, which legalize multi-wait instructions
    # for the TRN2 sync-struct limits.
    nc = bacc.Bacc(None, target_bir_lowering=False, debug=False)

    # ---- DRAM I/O (per-core shapes) ----
    # big3 = u [3, 4*4096] ++ w1eff [3, 512]  (one DMA -> one queue sem)
    big3_d = nc.dram_tensor("big3", [3, 4 * EDGES_PER_CORE + 512], dt,
                            kind="ExternalInput")
    # bigw = w2p [128,512] ++ w3cat(g2-folded) [128,1536] ++ ones/128 [128,1]
    bigw_d = nc.dram_tensor("bigw", [128, 2049], dt, kind="ExternalInput")
    # edata: per-edge chunk-major data, 102 f32/edge:
    #   xg0 0:16 | xg1 16:64 | b00 64:65 | b01 65:68 | b10 68:71 |
    #   b11 71:98 | wblk 98:102
    edata_d = nc.dram_tensor("edata", [128, NCHUNK * 102], dt,
                             kind="ExternalInput")
    out_d = nc.dram_tensor("outd", [4, NCHUNK * 64], dt, kind="ExternalOutput")

    # ---- preamble (raw bass, before Tile): load all constants, then an
    # all-engine barrier. Tile then sees no DMA producers, so compute
    # instructions never need DMA-queue waits (PE matmul allows only one
    # wait; multi-wait instructions fail walrus codegen).
    big3_t = nc.alloc_sbuf_tensor("big3_sb", [3, 4 * EDGES_PER_CORE + 512], dt)
    bigw_t = nc.alloc_sbuf_tensor("bigw_sb", [128, 2049], dt)
    edata_t = nc.alloc_sbuf_tensor("edata_sb", [128, NCHUNK * 102], dt)
    eps_t = nc.alloc_sbuf_tensor("eps_sb", [128, 1], dt)
    from contextlib import ExitStack
    es = ExitStack()
    ldsem = es.enter_context(nc.semaphore())
    nc.sync.dma_start(big3_t.ap(), big3_d[:]).then_inc(ldsem, 16)
    nc.sync.dma_start(bigw_t.ap(), bigw_d[:]).then_inc(ldsem, 16)
    nc.sync.dma_start(edata_t.ap(), edata_d[:]).then_inc(ldsem, 16)
    nc.gpsimd.memset(eps_t.ap(), EPS)
    for eng in (nc.tensor, nc.vector, nc.scalar, nc.gpsimd, nc.sync):
        eng.wait_ge(ldsem, 48)
    nc.all_engine_barrier()

    with tile.TileContext(nc, linearize=LINEARIZE) as tc:
        with (
            tc.tile_pool(name="const", bufs=1) as cpool,
            tc.tile_pool(name="work", bufs=3) as wpool,
            tc.tile_pool(name="small", bufs=4) as spool,
            tc.tile_pool(name="mlp", bufs=2, space=bass.MemorySpace.PSUM) as mlppool,
            tc.tile_pool(name="rps", bufs=1, space=bass.MemorySpace.PSUM) as rpool,
            tc.tile_pool(name="stat", bufs=1, space=bass.MemorySpace.PSUM) as statpool,
            tc.tile_pool(name="nout", bufs=2, space=bass.MemorySpace.PSUM) as npool,
        ):
            # ---- views onto the preamble-loaded constants ----
            outbuf = cpool.tile([4, NCHUNK * 64], dt)
            big3_sb = big3_t.ap()
            bigw_sb = bigw_t.ap()
            edata_sb = edata_t.ap().rearrange("p (a b) -> p a b", b=102)
            eps_sb = eps_t.ap()

            UOFF = 4 * EDGES_PER_CORE       # w1eff offset inside big3
            w3_sb = bigw_sb[:, 512:2048]
            ones_sb = bigw_sb[:, 2048:2049]   # holds 1/128
            xg0_sb = edata_sb[:, :, 0:16]
            xg1_sb = edata_sb[:, :, 16:64]
            b00_sb = edata_sb[:, :, 64:65]
            b01_sb = edata_sb[:, :, 65:68]
            b10_sb = edata_sb[:, :, 68:71]
            b11_sb = edata_sb[:, :, 71:98]
            wblk_sb = edata_sb[:, :, 98:102]



            for t in range(NTILE):
                e0 = t * 512
                # per-pair SBUF intermediates for this tile
                h2s = []
                r_all = spool.tile([128, 16], dt, tag="r_all")  # [P*4 + jj]
                for P in range(4):
                    psA = mlppool.tile([128, 512], dt, tag="ps")
                    psV = mlppool.tile([128, 512], dt, tag="ps")
                    h1 = wpool.tile([128, 512], dt, tag="h1")
                    sq = wpool.tile([128, 512], dt, tag="sq")
                    h2 = wpool.tile([128, 512], dt, tag=f"h2_{P}")
                    psVar = statpool.tile([128, 4], dt, tag="psVar")

                    # mm1: [3,128].T @ [3,512] -> [128,512]
                    nc.tensor.matmul(
                        psA[:],
                        big3_sb[:, UOFF + P * 128: UOFF + (P + 1) * 128],
                        big3_sb[:, P * EDGES_PER_CORE + e0:
                                P * EDGES_PER_CORE + e0 + 512],
                        start=True, stop=True,
                    )
                    nc.scalar.activation(h1[:], psA[:], AF.Relu)
                    # mm2 (w2 pre-centered): [128,128].T @ [128,512]
                    nc.tensor.matmul(
                        psV[:], bigw_sb[:, P * 128:(P + 1) * 128], h1[:],
                        start=True, stop=True,
                    )
                    # LN2 variance: square then per-chunk ones-matmul column sums
                    nc.scalar.activation(sq[:], psV[:], AF.Square)
                    for jj in range(4):
                        nc.tensor.matmul(
                            psVar[:, jj:jj + 1],
                            sq[:, jj * 128:(jj + 1) * 128],
                            ones_sb[:],
                            start=True, stop=True,
                        )
                    # r2 = rsqrt(var/128 + eps)  (edge-major [128,4])
                    sd = spool.tile([128, 4], dt, tag="sd")
                    nc.scalar.activation(sd[:], psVar[:], AF.Sqrt, bias=eps_sb[:])
                    if P == 0:
                        # fold basis00 per-edge scalar into pair00's r
                        rb = spool.tile([128, 4], dt, tag="rb")
                        nc.vector.reciprocal(rb[:], sd[:])
                        nc.vector.tensor_tensor(
                            r_all[:, 0:4], rb[:], b00_sb[:, t * 4:t * 4 + 4, 0],
                            op=ALU.mult,
                        )
                    else:
                        nc.vector.reciprocal(r_all[:, P * 4:(P + 1) * 4], sd[:])
                    # h2' = relu(v2c); g2 (>0) and r2 (>0) fold into w3 / k-mean
                    nc.scalar.activation(h2[:], psV[:], AF.Relu)
                    h2s.append(h2)

                # ---- per chunk: mm3 + einsum + masked k-mean ----
                for jj in range(4):
                    j = t * 4 + jj
                    psRa = rpool.tile([128, 512], dt, tag="psRa")   # R00|R01
                    psRb = rpool.tile([128, 1024], dt, tag="psRb")  # R11|R10
                    for P in range(2):
                        nc.tensor.matmul(
                            psRa[:, P * 256:(P + 1) * 256],
                            h2s[P][:, jj * 128:(jj + 1) * 128],
                            w3_sb[:, W3_OFF[P]:W3_OFF[P] + 256],
                            start=True, stop=True,
                        )
                    nc.tensor.matmul(
                        psRb[:, 0:512],
                        h2s[3][:, jj * 128:(jj + 1) * 128],
                        w3_sb[:, 768:1280],
                        start=True, stop=True,
                    )
                    nc.tensor.matmul(
                        psRb[:, 512:768],
                        h2s[3][:, jj * 128:(jj + 1) * 128],
                        w3_sb[:, 1280:1536],
                        start=True, stop=True,
                    )
                    nc.tensor.matmul(
                        psRb[:, 768:1024],
                        h2s[2][:, jj * 128:(jj + 1) * 128],
                        w3_sb[:, W3_OFF[2]:W3_OFF[2] + 256],
                        start=True, stop=True,
                    )

                    psN = npool.tile([4, 64], dt, tag="psN")

                    # stage f=1 R blocks PSUM->SBUF on ACT so GPSIMD can run
                    # their product passes (GPSIMD has no PSUM port); also
                    # releases psRa earlier for the next chunk's mm3.
                    ra = wpool.tile([128, 512], dt, tag="ra")
                    r10 = wpool.tile([128, 256], dt, tag="r10")
                    nc.scalar.copy(ra[:], psRa[:, 0:512])
                    nc.scalar.copy(r10[:], psRb[:, 768:1024])

                    # --- pairs 00 & 01 share: s[e, P, o] = sum_i R[e,(P,o,i)]*xg0[e,i]
                    pbuf = wpool.tile([128, 512], dt, tag="pbuf")
                    pbuf2 = wpool.tile([128, 256], dt, tag="pbuf2")
                    s01 = spool.tile([128, 32], dt, tag="s01")
                    xg0j = xg0_sb[:, j, :]  # [128, 16]
                    nc.gpsimd.tensor_tensor(
                        pbuf[:].rearrange("e (a i) -> e a i", i=16),
                        ra[:].rearrange("e (a i) -> e a i", i=16),
                        xg0j.unsqueeze(1).broadcast_to([128, 32, 16]),
                        op=ALU.mult,
                    )
                    nc.vector.tensor_reduce(
                        s01[:],
                        pbuf[:].rearrange("e (a i) -> e a i", i=16),
                        axis=AX.X, op=ALU.add,
                    )
                    # pair01 output: outP01[e, o, p] = s01[e, 16+o] * b01[e, p]
                    o01 = spool.tile([128, 48], dt, tag="o01")
                    nc.gpsimd.tensor_tensor(
                        o01[:].rearrange("e (o p) -> e o p", p=3),
                        s01[:, 16:32].unsqueeze(2).broadcast_to([128, 16, 3]),
                        b01_sb[:, j, :].unsqueeze(1).broadcast_to([128, 16, 3]),
                        op=ALU.mult,
                    )

                    # --- pair10: t10[e,i] = sum_q xg1[e,i,q]*b10[e,q]
                    tb = spool.tile([128, 48], dt, tag="tb")
                    t10 = spool.tile([128, 16], dt, tag="t10")
                    o10 = spool.tile([128, 16], dt, tag="o10")
                    nc.gpsimd.tensor_tensor(
                        tb[:].rearrange("e (i q) -> e i q", q=3),
                        xg1_sb[:, j, :].rearrange("e (i q) -> e i q", q=3),
                        b10_sb[:, j, :].unsqueeze(1).broadcast_to([128, 16, 3]),
                        op=ALU.mult,
                    )
                    nc.vector.tensor_reduce(
                        t10[:], tb[:].rearrange("e (i q) -> e i q", q=3),
                        axis=AX.X, op=ALU.add,
                    )
                    nc.gpsimd.tensor_tensor(
                        pbuf2[:].rearrange("e (o i) -> e o i", i=16),
                        r10[:].rearrange("e (o i) -> e o i", i=16),
                        t10[:].unsqueeze(1).broadcast_to([128, 16, 16]),
                        op=ALU.mult,
                    )
                    nc.vector.tensor_reduce(
                        o10[:], pbuf2[:].rearrange("e (o i) -> e o i", i=16),
                        axis=AX.X, op=ALU.add,
                    )

                    # --- pair11 ---
                    tbuf = wpool.tile([128, 432], dt, tag="tbuf")
                    t11 = spool.tile([128, 144], dt, tag="t11")
                    p11 = wpool.tile([128, 2304], dt, tag="p11")
                    o11 = spool.tile([128, 48], dt, tag="o11")
                    # T[e,(i,f,p)] = sum_q xg1[e,i,q] * b11[e,(p,q,f)]
                    # (split by p: HW APs allow at most 3 free dims)
                    tb5 = tbuf[:].rearrange("e (i f p q) -> e i f p q",
                                            f=3, p=3, q=3)
                    xg15 = (xg1_sb[:, j, :]
                            .rearrange("e (i q) -> e i q", q=3)
                            .unsqueeze(2)
                            .broadcast_to([128, 16, 3, 3]))
                    b115 = (b11_sb[:, j, :]
                            .rearrange("e (p q f) -> e p q f", q=3, f=3)
                            .transpose([0, 1, 3, 2]))  # -> [e, p, f, q]
                    for pp in range(3):
                        nc.gpsimd.tensor_tensor(
                            tb5[:, :, :, pp, :],
                            xg15,
                            b115[:, pp, :, :].unsqueeze(1)
                            .broadcast_to([128, 16, 3, 3]),
                            op=ALU.mult,
                        )
                    nc.vector.tensor_reduce(
                        t11[:],
                        tbuf[:].rearrange("e (ifp q) -> e ifp q", q=3),
                        axis=AX.X, op=ALU.add,
                    )
                    # prod5[e, o,p,i,f] = R11[e,(o,i,f)] * T[e,(i,f,p)]
                    # (split by p: HW APs allow at most 3 free dims)
                    p115 = p11[:].rearrange("e (o p i f) -> e o p i f",
                                            o=16, p=3, i=16)
                    r115 = psRb[:, 0:768].rearrange("e (o i f) -> e o i f",
                                                    o=16, i=16)
                    t115 = t11[:].rearrange("e (i f p) -> e i f p", i=16, f=3)
                    for pp in range(3):
                        nc.vector.tensor_tensor(
                            p115[:, :, pp, :, :],
                            r115,
                            t115[:, :, :, pp].unsqueeze(1)
                            .broadcast_to([128, 16, 16, 3]),
                            op=ALU.mult,
                        )
                    nc.vector.tensor_reduce(
                        o11[:],
                        p11[:].rearrange("e (op i_f) -> e op i_f", i_f=48),
                        axis=AX.X, op=ALU.add,
                    )

                    # --- masked k-mean via block-diag PE matmuls ---
                    # wr_all[e, P, nn] = wblk[e, nn] * r_all[e, P*4+jj]
                    wr = spool.tile([128, 4, 4], dt, tag="wr")
                    nc.vector.tensor_tensor(
                        wr[:],
                        wblk_sb[:, j, :].unsqueeze(1).broadcast_to([128, 4, 4]),
                        r_all[:].rearrange("e (P c) -> e P c", c=4)[:, :, jj:jj + 1]
                        .broadcast_to([128, 4, 4]),
                        op=ALU.mult,
                    )
                    # do=0: pairs 00 (s01[:, :16]) and 10
                    nc.tensor.matmul(psN[:, 0:16], wr[:, 0, :], s01[:, 0:16],
                                     start=True, stop=False)
                    nc.tensor.matmul(psN[:, 0:16], wr[:, 2, :], o10[:],
                                     start=False, stop=True)
                    # do=1: pairs 01 and 11
                    nc.tensor.matmul(psN[:, 16:64], wr[:, 1, :], o01[:],
                                     start=True, stop=False)
                    nc.tensor.matmul(psN[:, 16:64], wr[:, 3, :], o11[:],
                                     start=False, stop=True)
                    # ACT copy: keeps this off the DVE critical path (Bacc's
                    # generate_event_semaphores legalizes any multi-wait)
                    nc.scalar.copy(outbuf[:, j * 64:(j + 1) * 64], psN[:])

            nc.sync.dma_start(out_d[:], outbuf[:])

    es.close()
    nc.compile()
    return nc


def _prep_core_inputs(x0, x1, rel_dist, basis, params, nidx, nmask):
    """Host-side folding + packing. Returns list of 8 in_maps."""
    f32 = np.float32
    # --- per-pair folded weights (shared across cores) ---
    w1eff = np.zeros((3, 4 * 128), f32)
    w2p = np.zeros((128, 4 * 128), f32)
    g2p = np.zeros((128, 4), f32)
    w3cat = np.zeros((128, 1536), f32)
    ones_col = np.full((128, 1), 1.0 / 128.0, f32)
    lnco = []  # (m20, m11, m02, mw, mb) per pair for u computation
    for P, pk in enumerate(PAIR_KEYS):
        rp = {k: np.asarray(v, f32) for k, v in params["rp"][pk].items()}
        w1 = rp["w1"][0]          # [128]
        b1 = rp["b1"]
        mw, mb = w1.mean(), b1.mean()
        wc, bc = w1 - mw, b1 - mb
        m20 = (wc * wc).mean()
        m11 = (wc * bc).mean()
        m02 = (bc * bc).mean()
        lnco.append((m20, m11, m02))
        w1eff[0, P * 128:(P + 1) * 128] = wc * rp["g1"]
        w1eff[1, P * 128:(P + 1) * 128] = bc * rp["g1"]
        w1eff[2, P * 128:(P + 1) * 128] = rp["be1"]
        w2p[:, P * 128:(P + 1) * 128] = rp["w2"] - rp["w2"].mean(1, keepdims=True)
        g2p[:, P] = rp["g2"]
        # w3 natural layout [128, o*i*f] == [c, (o,i,f)]; g2 (>0) folded in rows
        w3cat[:, W3_OFF[P]:W3_OFF[P] + W3_N[P]] = rp["g2"][:, None] * rp["w3"]
    # fast-path validity
    fast = True
    for pk in PAIR_KEYS:
        rp = params["rp"][pk]
        b2c = np.asarray(rp["b2"], f32)
        b2c = b2c - b2c.mean()
        if (np.abs(b2c).max() > 0 or np.abs(np.asarray(rp["be2"])).max() > 0
                or np.abs(np.asarray(rp["b3"])).max() > 0
                or np.asarray(rp["g2"]).min() <= 0):
            fast = False
    if not fast:
        return None

    nidx = np.asarray(nidx)
    nmask_f = np.asarray(nmask).astype(f32)
    denom = np.clip(nmask_f.sum(2), 1e-5, None)      # [B, N]
    wtil = nmask_f / denom[:, :, None]               # [B, N, K]

    in_maps = []
    for c in range(NCORES):
        b = c // 2
        n0 = (c % 2) * NODES_PER_CORE
        ns = slice(n0, n0 + NODES_PER_CORE)
        rd = np.asarray(rel_dist[b, ns], f32)        # [128, 32]
        idx = nidx[b, ns]                            # [128, 32]
        # edge order: e = n_local*32 + k ; chunk j = e//128 ; lane = e%128
        x = rd.reshape(-1)                           # [4096]
        u = np.zeros((3, 4 * EDGES_PER_CORE), f32)
        for P in range(4):
            m20, m11, m02 = lnco[P]
            r1 = 1.0 / np.sqrt(x * x * m20 + 2 * x * m11 + m02 + EPS)
            u[0, P * EDGES_PER_CORE:(P + 1) * EDGES_PER_CORE] = x * r1
            u[1, P * EDGES_PER_CORE:(P + 1) * EDGES_PER_CORE] = r1
            u[2, P * EDGES_PER_CORE:(P + 1) * EDGES_PER_CORE] = 1.0

        flat_idx = idx.reshape(-1)                   # [4096] node ids
        xg0 = np.asarray(x0[b], f32)[flat_idx, :, 0]          # [4096, 16]
        xg1 = np.asarray(x1[b], f32)[flat_idx].reshape(-1, 48)  # [4096, 16*3]
        bas = {pk: np.asarray(basis[pk][b, ns], f32).reshape(EDGES_PER_CORE, -1)
               for pk in PAIR_KEYS}
        wt = wtil[b, ns].reshape(-1)                 # [4096]
        # block-diag k-mean weights: wblk[e, nn] = wt[e] if (e%128)//32==nn
        wblk = np.zeros((EDGES_PER_CORE, 4), f32)
        lane = np.arange(EDGES_PER_CORE) % 128
        wblk[np.arange(EDGES_PER_CORE), lane // 32] = wt

        def pack(a):  # [4096, F] -> [128, NCHUNK*F] with [lane, chunk, F]
            F = a.shape[1]
            return np.ascontiguousarray(
                a.reshape(NCHUNK, 128, F).transpose(1, 0, 2).reshape(128, NCHUNK * F)
            )

        edata = np.concatenate(
            [xg0, xg1, bas["0,0"], bas["0,1"], bas["1,0"], bas["1,1"], wblk],
            axis=1)  # [4096, 102]
        in_maps.append({
            "big3": np.concatenate([u, w1eff], axis=1),
            "bigw": np.concatenate([w2p, w3cat, ones_col], axis=1),
            "edata": pack(edata),
        })
    return in_maps


def kernel(x0, x1, rel_dist, basis00, basis01, basis10, basis11, params,
           neighbor_indices, neighbor_masks):
    x0 = np.asarray(x0, np.float32)
    x1 = np.asarray(x1, np.float32)
    basis = {"0,0": np.asarray(basis00), "0,1": np.asarray(basis01),
             "1,0": np.asarray(basis10), "1,1": np.asarray(basis11)}

    in_maps = _prep_core_inputs(x0, x1, rel_dist, basis, params,
                                neighbor_indices, neighbor_masks)
    if in_maps is None:  # general-params fallback (never taken for this module)
        return _host_forward(x0, x1, np.asarray(rel_dist, np.float32), basis,
                             params, np.asarray(neighbor_indices),
                             np.asarray(neighbor_masks))

    if "nc" not in _CACHE:
        _CACHE["nc"] = _build_bass()
    nc = _CACHE["nc"]

    from concourse import bass_utils
    res = bass_utils.run_bass_kernel_spmd(nc, in_maps, core_ids=list(range(NCORES)))
    outs = res.results

    # reassemble: outd [4, NCHUNK*64] -> per core nodes
    out0 = np.zeros((B, N, 16), np.float32)
    out1 = np.zeros((B, N, 48), np.float32)
    for c in range(NCORES):
        od = outs[c]["outd"].reshape(4, NCHUNK, 64)   # [nn, chunk, 64]
        b = c // 2
        n0 = (c % 2) * NODES_PER_CORE
        # node = n0 + chunk*4 + nn
        o = od.transpose(1, 0, 2)                     # [chunk, nn, 64]
        o = o.reshape(NODES_PER_CORE, 64)
        out0[b, n0:n0 + NODES_PER_CORE] = o[:, 0:16]
        out1[b, n0:n0 + NODES_PER_CORE] = o[:, 16:64]

    out0 = out0.reshape(B, N, 16, 1)
    out1 = out1.reshape(B, N, 16, 3)
    # self-interaction (host, tiny)
    w_si0 = np.asarray(params["w_si0"], np.float32)
    w_si1 = np.asarray(params["w_si1"], np.float32)
    out0 = out0 + np.einsum("bndm,de->bnem", x0, w_si0)
    out1 = out1 + np.einsum("bndm,de->bnem", x1, w_si1)
    return out0.astype(np.float32), out1.astype(np.float32)
